# revision 1
# baseline (speedup 1.0000x reference)
"""Multi-head causal attention with RoPE on 8 TRN2 NeuronCores.

Tensor-parallel over heads: core c computes heads (2c, 2c+1).
  Phase 1: Q^T,K^T (with RoPE) and V projections from pre-transposed x.
  Phase 2: causal attention per (batch, head) in transposed orientation
           (scores^T = K^T_blk^T @ Q^T), softmax without max-subtraction
           (scores are O(1) here), softmax denominators via ones-matmul
           over the partition axis.
  Phase 3: per-batch AllToAll redistributes context from head-sharded to
           sequence-sharded; each core applies the FULL Wo to its 256-token
           slice per batch: out^T slice = Wo^T @ ctx_full^T[:, t_slice].
Host does layout prep (x transpose, RoPE tables, causal mask tiles) and
final unshard (interleave per-core token slices).

TensorEngine operands are float32r (~2^-13 rounding, 4x faster than fp32
matmul) except the final Wo stage which runs bf16 (halves the collective
and its DMA); accumulation is always fp32 in PSUM.
"""
import ml_dtypes
import numpy as np

import concourse.bass as bass  # noqa: F401  (engine namespaces live on nc)
import concourse.mybir as mybir
import concourse.tile as tile
from concourse import bacc
from concourse import bass_utils

B, T, DM, H, D = 2, 2048, 2048, 16, 128
NCORES = 8
HPC = H // NCORES        # heads per core
DLOC = HPC * D           # local head width (256)
BT = B * T               # 4096 token rows
P = 128
TCH = 512                # free-dim chunk
NKB = DM // P            # 16 contraction blocks
NTB = T // P             # 16 token blocks per batch
NBCH = T // TCH          # 4 token chunks per batch
TSL = T // NCORES        # 256-token output slice per core per batch
SCALE = 1.0 / float(np.sqrt(D))
F32 = mybir.dt.float32
F32R = mybir.dt.float32r
BF16 = mybir.dt.bfloat16
MUL = mybir.AluOpType.mult
ADD = mybir.AluOpType.add

_nc_cache = None


def _build():
    nc = bacc.Bacc("TRN2", target_bir_lowering=False, debug=False,
                   num_devices=NCORES)
    xt = nc.dram_tensor("xt", [DM, BT], F32R, kind="ExternalInput")
    wq = nc.dram_tensor("wq", [DM, DLOC], F32R, kind="ExternalInput")
    wk = nc.dram_tensor("wk", [DM, DLOC], F32R, kind="ExternalInput")
    wv = nc.dram_tensor("wv", [DM, DLOC], F32R, kind="ExternalInput")
    wo = nc.dram_tensor("wo", [DM, DM], BF16, kind="ExternalInput")
    cf = nc.dram_tensor("cf", [P, T], F32, kind="ExternalInput")
    sf = nc.dram_tensor("sf", [P, T], F32, kind="ExternalInput")
    cm = nc.dram_tensor("cm", [P, 4 * TCH], BF16, kind="ExternalInput")
    onec = nc.dram_tensor("onec", [P, 1], BF16, kind="ExternalInput")
    oner = nc.dram_tensor("oner", [1, P], F32R, kind="ExternalInput")
    # out^T slice: [out_cols, b0 slice | b1 slice]
    outT = nc.dram_tensor("out", [DM, B * TSL], F32, kind="ExternalOutput")

    with tile.TileContext(nc) as tc:
        with tc.tile_pool(name="dram", bufs=1, space="DRAM") as dpool, \
             tc.tile_pool(name="const", bufs=1) as cpool, \
             tc.tile_pool(name="pre2", bufs=1) as prepool:
            qT_d = [dpool.tile([DLOC, T], F32R, name=f"qT{b}") for b in range(B)]
            kT_d = [dpool.tile([DLOC, T], F32R, name=f"kT{b}") for b in range(B)]
            v_d = [dpool.tile([T, DLOC], BF16, name=f"v{b}") for b in range(B)]
            # A2A input: 8 rank-blocks x [256 local hd, 256 t-slice]
            ctxA_d = [dpool.tile([DM, TSL], BF16, name=f"ctxA{b}") for b in range(B)]
            # A2A output: stacked = ctx_full^T [2048 hd, my 256 t]
            gout_d = [dpool.tile([DM, TSL], BF16, name=f"gout{b}") for b in range(B)]

            cf_s = cpool.tile([P, T], F32)
            sf_s = cpool.tile([P, T], F32)
            cm_s = cpool.tile([P, 4 * TCH], BF16)
            onec_s = cpool.tile([P, 1], BF16)
            oner_s = cpool.tile([1, P], F32R)
            pre_kT = prepool.tile([P, T], F32R, name="pre_kT")
            pre_v = prepool.tile([P, NTB, D], BF16, name="pre_v")
            pre_qT = prepool.tile([P, TCH], F32R, name="pre_qT")
            bar_in = dpool.tile([8, 4], F32)
            bar_out = dpool.tile([64, 4], F32, addr_space="Shared")
            nc.sync.dma_start(cf_s[:], cf.ap())
            nc.sync.dma_start(sf_s[:], sf.ap())
            nc.sync.dma_start(cm_s[:], cm.ap())
            nc.sync.dma_start(onec_s[:], onec.ap())
            nc.sync.dma_start(oner_s[:], oner.ap())
            # start-skew absorber: cores align here while phase 1 computes
            nc.sync.dma_start(bar_in[:], cf.ap()[0:8, 0:4])
            nc.gpsimd.collective_compute(
                "AllGather", mybir.AluOpType.bypass,
                replica_groups=[list(range(NCORES))],
                ins=[bar_in[:].opt()], outs=[bar_out[:].opt()])

            # ---------------- Phase 1: projections + RoPE ----------------
            with tc.tile_pool(name="p1w", bufs=1) as wpool, \
                 tc.tile_pool(name="p1", bufs=2) as pool, \
                 tc.tile_pool(name="ps1", bufs=2, space="PSUM") as ps1:
                wq_s = wpool.tile([P, NKB, DLOC], F32R)
                wk_s = wpool.tile([P, NKB, DLOC], F32R)
                wv_s = wpool.tile([P, NKB, DLOC], F32R)
                nc.sync.dma_start(wq_s[:], wq.ap().rearrange("(kb p) m -> p kb m", p=P))
                nc.sync.dma_start(wk_s[:], wk.ap().rearrange("(kb p) m -> p kb m", p=P))
                nc.sync.dma_start(wv_s[:], wv.ap().rearrange("(kb p) m -> p kb m", p=P))

                XCH = TCH
                for i in range(BT // XCH):
                    bb, ic = i // (T // XCH), i % (T // XCH)
                    xt_t = pool.tile([P, NKB, XCH], F32R, tag="xt")
                    nc.sync.dma_start(
                        xt_t[:],
                        xt.ap()[:, i * XCH:(i + 1) * XCH]
                        .rearrange("(kb p) n -> p kb n", p=P))
                    cs = cf_s[:, ic * XCH:(ic + 1) * XCH]
                    sn = sf_s[:, ic * XCH:(ic + 1) * XCH]
                    for w_s, dst in ((wq_s, qT_d), (wk_s, kT_d)):
                        for m in range(HPC):
                            ps = ps1.tile([P, XCH], F32, tag="qk")
                            for kb in range(NKB):
                                nc.tensor.matmul(
                                    ps[:], w_s[:, kb, m * P:(m + 1) * P],
                                    xt_t[:, kb],
                                    start=(kb == 0), stop=(kb == NKB - 1))
                            # RoPE: rq = q*cos_full + rot(q)*sin_signed
                            tmp = pool.tile([P, XCH], F32, tag="tmp")
                            tmp2 = pool.tile([P, XCH], F32, tag="tmp2")
                            rq = pool.tile([P, XCH], F32R, tag="rq")
                            nc.vector.tensor_tensor(tmp[0:64], ps[64:128], sn[0:64], MUL)
                            nc.vector.tensor_tensor(tmp[64:128], ps[0:64], sn[64:128], MUL)
                            nc.vector.tensor_tensor(tmp2[:], ps[:], cs, MUL)
                            nc.vector.tensor_tensor(rq[:], tmp2[:], tmp[:], ADD)
                            nc.sync.dma_start(
                                dst[bb][m * P:(m + 1) * P, ic * XCH:(ic + 1) * XCH],
                                rq[:])
                    for tb in range(XCH // P):
                        psv = ps1.tile([P, DLOC], F32, tag="v")
                        for kb in range(NKB):
                            nc.tensor.matmul(
                                psv[:], xt_t[:, kb, tb * P:(tb + 1) * P],
                                wv_s[:, kb],
                                start=(kb == 0), stop=(kb == NKB - 1))
                        vsb = pool.tile([P, DLOC], BF16, tag="vsb")
                        nc.vector.tensor_copy(vsb[:], psv[:])
                        r0 = ic * XCH + tb * P
                        nc.sync.dma_start(v_d[bb][r0:r0 + P, :], vsb[:])

            # wo_s pool opens after phase-1 pools close, so its 8MB
            # reuses phase-1 SBUF and the load runs during attention.
            with tc.tile_pool(name="p3w", bufs=1) as wpool3:
                # ---------------- Phase 2: causal attention + A2A ----------------
                with tc.tile_pool(name="p2", bufs=3) as pool2, \
                     tc.tile_pool(name="p2t", bufs=6) as ppool, \
                     tc.tile_pool(name="ps_s", bufs=2, space="PSUM") as ps_sp, \
                     tc.tile_pool(name="ps_acc", bufs=2, space="PSUM") as ps_accp, \
                     tc.tile_pool(name="ps_misc", bufs=1, space="PSUM") as ps_mp:
                    wo_s = wpool3.tile([P, NKB, DM], BF16)
                    for b in range(B):
                        for hl in range(HPC):
                            first = (b == 0 and hl == 0)
                            if first:
                                kT_s, v_s = pre_kT, pre_v
                            else:
                                kT_s = pool2.tile([P, T], F32R, tag="kT")
                                v_s = pool2.tile([P, NTB, D], BF16, tag="v")
                            nc.sync.dma_start(kT_s[:], kT_d[b][hl * P:(hl + 1) * P, :])
                            nc.sync.dma_start(
                                v_s[:],
                                v_d[b][:, hl * D:(hl + 1) * D]
                                .rearrange("(j p) d -> p j d", p=P))
                            if first:
                                # wo loads queue behind the first section's loads
                                for kb in range(NKB):
                                    nc.sync.dma_start(
                                        wo_s[:, kb],
                                        wo.ap()[kb * P:(kb + 1) * P, :])
                            for cq in range(NBCH):
                                if first and cq == 0:
                                    qT_s = pre_qT
                                else:
                                    qT_s = pool2.tile([P, TCH], F32R, tag="qT")
                                nc.sync.dma_start(
                                    qT_s[:],
                                    qT_d[b][hl * P:(hl + 1) * P,
                                            cq * TCH:(cq + 1) * TCH])
                                nblk = 4 * cq + 4
                                ps_ctx = ps_accp.tile([P, TCH], F32, tag="ctx")
                                ps_sum = ps_mp.tile([1, TCH], F32, tag="sum")
                                for jp in range(nblk // 2):
                                    j0 = 2 * jp
                                    ps_sc = ps_sp.tile([P, 2, TCH], F32, tag="s")
                                    nc.tensor.matmul(
                                        ps_sc[:, 0], kT_s[:, j0 * P:(j0 + 1) * P],
                                        qT_s[:], start=True, stop=True)
                                    nc.tensor.matmul(
                                        ps_sc[:, 1], kT_s[:, (j0 + 1) * P:(j0 + 2) * P],
                                        qT_s[:], start=True, stop=True)
                                    pT = ppool.tile([P, 2, TCH], BF16, tag="pT")
                                    nc.scalar.activation(
                                        pT[:], ps_sc[:],
                                        mybir.ActivationFunctionType.Exp, scale=SCALE)
                                    vmask = j0 - 4 * cq
                                    if vmask >= 0:
                                        nc.vector.tensor_tensor(
                                            pT[:], pT[:],
                                            cm_s[:, vmask * TCH:(vmask + 2) * TCH]
                                            .rearrange("p (v n) -> p v n", v=2), MUL)
                                    for h in range(2):
                                        j = j0 + h
                                        nc.tensor.matmul(
                                            ps_ctx[:], v_s[:, j], pT[:, h],
                                            start=(j == 0), stop=(j == nblk - 1))
                                        nc.tensor.matmul(
                                            ps_sum[:], onec_s[:], pT[:, h],
                                            start=(j == 0), stop=(j == nblk - 1))
                                rs = pool2.tile([1, TCH], F32R, tag="rs")
                                nc.vector.tensor_copy(rs[:], ps_sum[:])
                                ps_bc = ps_mp.tile([P, TCH], F32, tag="bc")
                                nc.tensor.matmul(ps_bc[:], oner_s[:], rs[:],
                                                 start=True, stop=True)
                                bc_s = pool2.tile([P, TCH], F32, tag="bc_s")
                                with nc.allow_low_precision(reason="plain elementwise recip"):
                                    nc.vector.reciprocal(bc_s[:], ps_bc[:])
                                ctx_s = pool2.tile([P, TCH], BF16, tag="ctx")
                                nc.vector.tensor_tensor(ctx_s[:], ps_ctx[:], bc_s[:], MUL)
                                # scatter the 512-token chunk into two rank blocks
                                nc.sync.dma_start(
                                    ctxA_d[b]
                                    .rearrange("(r q p) n -> q p r n", q=HPC, p=P)
                                    [hl, :, 2 * cq:2 * cq + 2],
                                    ctx_s.rearrange("p (r n) -> p r n", r=2))
                        nc.gpsimd.collective_compute(
                            "AllToAll", mybir.AluOpType.bypass,
                            replica_groups=[list(range(NCORES))],
                            ins=[ctxA_d[b][:].opt()],
                            outs=[gout_d[b][:].opt()])

                # ---------------- Phase 3: output projection (full Wo) -----------
                with tc.tile_pool(name="p3", bufs=2) as pool3, \
                     tc.tile_pool(name="ps3", bufs=2, space="PSUM") as ps3, \
                     tc.tile_wait_until(1):
                    for b in range(B):
                        g_t = pool3.tile([P, NKB, TSL], BF16, tag="g")
                        if b == 0:
                            # token write: forces this load (and hence Wo-b0 matmuls)
                            # after the last batch's ctx is complete, so Wo-b0 fills
                            # the PE hole while A2A-b1 is in flight instead of being
                            # greedily scheduled into the attention tail.
                            nc.sync.dma_start(g_t[:1, 0, :4], ctxA_d[B - 1][1920:1921, 0:4])
                        else:
                            # chain behind Wo-b0's final output so the sync queue
                            # never parks this load's A2A wait ahead of Wo-b0.
                            nc.sync.dma_start(g_t[:1, 0, :4],
                                              outT.ap()[1920:1921, 0:4].bitcast(BF16)[:, 0:4])
                        nc.sync.dma_start(
                            g_t[:], gout_d[b][:].rearrange("(kb p) n -> p kb n", p=P))
                        for m in range(DM // P):
                            pso = ps3.tile([P, TSL], F32, tag="o")
                            for kb in range(NKB):
                                nc.tensor.matmul(
                                    pso[:], wo_s[:, kb, m * P:(m + 1) * P], g_t[:, kb],
                                    start=(kb == 0), stop=(kb == NKB - 1))
                            o_s = pool3.tile([P, TSL], F32, tag="o_s")
                            nc.vector.tensor_copy(o_s[:], pso[:])
                            nc.sync.dma_start(
                                outT.ap()[m * P:(m + 1) * P, b * TSL:(b + 1) * TSL],
                                o_s[:])

    nc.compile()
    return nc


def _prep_inputs(x, cos, sin, Wq, Wk, Wv, Wo):
    x = np.asarray(x, dtype=np.float32)
    cos = np.asarray(cos, dtype=np.float32)
    sin = np.asarray(sin, dtype=np.float32)
    xt = np.ascontiguousarray(x.reshape(BT, DM).T)
    cf = np.empty((P, T), np.float32)
    cf[:64] = cos.T
    cf[64:] = cos.T
    sf = np.empty((P, T), np.float32)
    sf[:64] = -sin.T
    sf[64:] = sin.T
    qq = np.arange(TCH, dtype=np.int64)[None, :]
    rr = np.arange(P, dtype=np.int64)[:, None]
    cm = np.concatenate(
        [(qq >= v * P + rr).astype(np.float32) for v in range(TCH // P)],
        axis=1).astype(ml_dtypes.bfloat16)
    onec = np.ones((P, 1), np.float32).astype(ml_dtypes.bfloat16)
    oner = np.ones((1, P), np.float32)
    wo_full = np.ascontiguousarray(np.asarray(Wo, np.float32)).astype(ml_dtypes.bfloat16)
    in_maps = []
    for c in range(NCORES):
        sl = slice(c * DLOC, (c + 1) * DLOC)
        in_maps.append({
            "xt": xt, "cf": cf, "sf": sf, "cm": cm,
            "onec": onec, "oner": oner,
            "wq": np.ascontiguousarray(np.asarray(Wq, np.float32)[:, sl]),
            "wk": np.ascontiguousarray(np.asarray(Wk, np.float32)[:, sl]),
            "wv": np.ascontiguousarray(np.asarray(Wv, np.float32)[:, sl]),
            "wo": wo_full,
        })
    return in_maps


def run(x, mask, cos, sin, Wq, Wk, Wv, Wo, trace=False):
    global _nc_cache
    if _nc_cache is None:
        _nc_cache = _build()
    in_maps = _prep_inputs(x, cos, sin, Wq, Wk, Wv, Wo)
    res = bass_utils.run_bass_kernel_spmd(
        _nc_cache, in_maps, core_ids=list(range(NCORES)), trace=trace)
    out = np.empty((B, T, DM), np.float32)
    for c in range(NCORES):
        o = res.results[c]["out"]  # [DM, B*TSL]
        for b in range(B):
            out[b, c * TSL:(c + 1) * TSL, :] = o[:, b * TSL:(b + 1) * TSL].T
    return out, res


def kernel(x, mask, cos, sin, Wq, Wk, Wv, Wo):
    out, _ = run(x, mask, cos, sin, Wq, Wk, Wv, Wo, trace=False)
    return out



# revision 3
# speedup vs baseline: 1.0279x; 1.0279x over previous
"""Multi-head causal attention with RoPE on 8 TRN2 NeuronCores.

Tensor-parallel over heads: core c computes heads (2c, 2c+1).
  Phase 1: Q^T,K^T (with RoPE) and V projections from pre-transposed x
           (bf16 inputs/weights, fp32 PSUM accumulate).
  Phase 2: causal attention per (batch, head) in transposed orientation
           (scores^T = K^T_blk^T @ Q^T), softmax without max-subtraction.
           Softmax denominators: exp tiles are accumulated elementwise on
           the Pool/Vector engines into S, then a single ones-matmul per
           512-query chunk reduces S over the partition axis (instead of
           one ones-matmul per key block, which wasted ~12% of PE time).
  Phase 3: per (batch, head) AllToAll (4 small collectives instead of 2
           big ones) redistributes context from head-sharded to
           sequence-sharded while attention/Wo compute continues; each
           core applies the FULL Wo to its 256-token slice per batch.
Two DMA trigger queues are used: the Sync HWDGE queue carries the
fine-grained flow (section loads, ctx scatters, outputs) while the
Activation HWDGE queue carries bulk prefetches (weights, x chunks, Wo,
post-collective gathers) so a waiting bulk load never head-of-line
blocks a latency-critical load.
"""
import ml_dtypes
import numpy as np

import concourse.bass as bass  # noqa: F401  (engine namespaces live on nc)
import concourse.mybir as mybir
import concourse.tile as tile
from concourse import bacc
from concourse import bass_utils

B, T, DM, H, D = 2, 2048, 2048, 16, 128
NCORES = 8
HPC = H // NCORES        # heads per core
DLOC = HPC * D           # local head width (256)
BT = B * T               # 4096 token rows
P = 128
TCH = 512                # free-dim chunk
NKB = DM // P            # 16 contraction blocks
NTB = T // P             # 16 token blocks per batch
NBCH = T // TCH          # 4 token chunks per batch
TSL = T // NCORES        # 256-token output slice per core per batch
SCALE = 1.0 / float(np.sqrt(D))
F32 = mybir.dt.float32
F32R = mybir.dt.float32r
BF16 = mybir.dt.bfloat16
MUL = mybir.AluOpType.mult
ADD = mybir.AluOpType.add
EXP = mybir.ActivationFunctionType.Exp

_nc_cache = None


def _build():
    nc = bacc.Bacc("TRN2", target_bir_lowering=False, debug=False,
                   num_devices=NCORES)
    xt = nc.dram_tensor("xt", [DM, BT], BF16, kind="ExternalInput")
    wq = nc.dram_tensor("wq", [DM, DLOC], BF16, kind="ExternalInput")
    wk = nc.dram_tensor("wk", [DM, DLOC], BF16, kind="ExternalInput")
    wv = nc.dram_tensor("wv", [DM, DLOC], BF16, kind="ExternalInput")
    wo = nc.dram_tensor("wo", [DM, DM], BF16, kind="ExternalInput")
    cf = nc.dram_tensor("cf", [P, T], F32, kind="ExternalInput")
    sf = nc.dram_tensor("sf", [P, T], F32, kind="ExternalInput")
    cm = nc.dram_tensor("cm", [P, 4 * TCH], BF16, kind="ExternalInput")
    onec = nc.dram_tensor("onec", [P, 1], F32R, kind="ExternalInput")
    oner = nc.dram_tensor("oner", [1, P], F32R, kind="ExternalInput")
    # out^T slice: [out_cols, b0 slice | b1 slice]
    outT = nc.dram_tensor("out", [DM, B * TSL], F32, kind="ExternalOutput")

    with tile.TileContext(nc) as tc:
        with tc.tile_pool(name="dram", bufs=1, space="DRAM") as dpool, \
             tc.tile_pool(name="const", bufs=1) as cpool, \
             tc.tile_pool(name="pre2", bufs=1) as prepool:
            qT_d = [dpool.tile([DLOC, T], F32R, name=f"qT{b}") for b in range(B)]
            kT_d = [dpool.tile([DLOC, T], F32R, name=f"kT{b}") for b in range(B)]
            v_d = [dpool.tile([T, DLOC], BF16, name=f"v{b}") for b in range(B)]
            # A2A per (batch, local head): 8 rank-blocks x [128 hd, 256 t-slice]
            ctxA_d = [[dpool.tile([NCORES * P, TSL], BF16, name=f"ctxA{b}_{hl}")
                       for hl in range(HPC)] for b in range(B)]
            gout_d = [[dpool.tile([NCORES * P, TSL], BF16, name=f"gout{b}_{hl}")
                       for hl in range(HPC)] for b in range(B)]

            cf_s = cpool.tile([P, T], F32)
            sf_s = cpool.tile([P, T], F32)
            cm_s = cpool.tile([P, 4 * TCH], BF16)
            onec_s = cpool.tile([P, 1], F32R)
            oner_s = cpool.tile([1, P], F32R)
            pre_kT = prepool.tile([P, T], F32R, name="pre_kT")
            pre_v = prepool.tile([P, NTB, D], BF16, name="pre_v")
            pre_qT = prepool.tile([P, TCH], F32R, name="pre_qT")
            bar_in = dpool.tile([8, 4], F32)
            bar_out = dpool.tile([64, 4], F32, addr_space="Shared")
            # start-skew absorber: cores align here while phase 1 computes
            nc.sync.dma_start(bar_in[:], cf.ap()[0:8, 0:4])
            nc.gpsimd.collective_compute(
                "AllGather", mybir.AluOpType.bypass,
                replica_groups=[list(range(NCORES))],
                ins=[bar_in[:].opt()], outs=[bar_out[:].opt()])
            nc.sync.dma_start(cf_s[:], cf.ap())
            nc.sync.dma_start(sf_s[:], sf.ap())
            nc.sync.dma_start(cm_s[:], cm.ap())
            nc.sync.dma_start(onec_s[:], onec.ap())
            nc.sync.dma_start(oner_s[:], oner.ap())

            # ---------------- Phase 1: projections + RoPE ----------------
            with tc.tile_pool(name="p1w", bufs=1) as wpool, \
                 tc.tile_pool(name="p1", bufs=2) as pool, \
                 tc.tile_pool(name="ps1", bufs=2, space="PSUM") as ps1:
                wq_s = wpool.tile([P, NKB, DLOC], BF16)
                wk_s = wpool.tile([P, NKB, DLOC], BF16)
                wv_s = wpool.tile([P, NKB, DLOC], BF16)
                # wq + first x chunk go first on the bulk queue so the
                # first matmul can start ~15us in; wk/wv follow behind.
                nc.scalar.dma_start(wq_s[:], wq.ap().rearrange("(kb p) m -> p kb m", p=P))

                XCH = TCH
                for i in range(BT // XCH):
                    bb, ic = i // (T // XCH), i % (T // XCH)
                    xt_t = pool.tile([P, NKB, XCH], BF16, tag="xt")
                    nc.scalar.dma_start(
                        xt_t[:],
                        xt.ap()[:, i * XCH:(i + 1) * XCH]
                        .rearrange("(kb p) n -> p kb n", p=P))
                    if i == 0:
                        nc.scalar.dma_start(
                            wk_s[:], wk.ap().rearrange("(kb p) m -> p kb m", p=P))
                        nc.scalar.dma_start(
                            wv_s[:], wv.ap().rearrange("(kb p) m -> p kb m", p=P))
                    cs = cf_s[:, ic * XCH:(ic + 1) * XCH]
                    sn = sf_s[:, ic * XCH:(ic + 1) * XCH]
                    for w_s, dst in ((wq_s, qT_d), (wk_s, kT_d)):
                        for m in range(HPC):
                            ps = ps1.tile([P, XCH], F32, tag="qk")
                            for kb in range(NKB):
                                nc.tensor.matmul(
                                    ps[:], w_s[:, kb, m * P:(m + 1) * P],
                                    xt_t[:, kb],
                                    start=(kb == 0), stop=(kb == NKB - 1))
                            # RoPE: rq = q*cos_full + rot(q)*sin_signed
                            tmp = pool.tile([P, XCH], F32, tag="tmp")
                            tmp2 = pool.tile([P, XCH], F32, tag="tmp2")
                            rq = pool.tile([P, XCH], F32R, tag="rq")
                            nc.vector.tensor_tensor(tmp[0:64], ps[64:128], sn[0:64], MUL)
                            nc.vector.tensor_tensor(tmp[64:128], ps[0:64], sn[64:128], MUL)
                            nc.vector.tensor_tensor(tmp2[:], ps[:], cs, MUL)
                            nc.vector.tensor_tensor(rq[:], tmp2[:], tmp[:], ADD)
                            nc.sync.dma_start(
                                dst[bb][m * P:(m + 1) * P, ic * XCH:(ic + 1) * XCH],
                                rq[:])
                    for tb in range(XCH // P):
                        psv = ps1.tile([P, DLOC], F32, tag="v")
                        for kb in range(NKB):
                            nc.tensor.matmul(
                                psv[:], xt_t[:, kb, tb * P:(tb + 1) * P],
                                wv_s[:, kb],
                                start=(kb == 0), stop=(kb == NKB - 1))
                        vsb = pool.tile([P, DLOC], BF16, tag="vsb")
                        nc.vector.tensor_copy(vsb[:], psv[:])
                        r0 = ic * XCH + tb * P
                        nc.sync.dma_start(v_d[bb][r0:r0 + P, :], vsb[:])

                # prefetch attention section 1 inputs on the bulk queue
                nc.scalar.dma_start(pre_kT[:], kT_d[0][0:P, :])
                nc.scalar.dma_start(
                    pre_v[:],
                    v_d[0][:, 0:D].rearrange("(j p) d -> p j d", p=P))
                nc.scalar.dma_start(pre_qT[:], qT_d[0][0:P, 0:TCH])

            # wo_s pool opens after phase-1 pools close, so its 8MB
            # reuses phase-1 SBUF and the load runs during attention.
            with tc.tile_pool(name="p3w", bufs=1) as wpool3:
                # ------------- Phase 2: causal attention + split A2A -------------
                with tc.tile_pool(name="p2", bufs=3) as pool2, \
                     tc.tile_pool(name="p2s", bufs=2) as spool, \
                     tc.tile_pool(name="p2t", bufs=6) as ppool, \
                     tc.tile_pool(name="ps_s", bufs=2, space="PSUM") as ps_sp, \
                     tc.tile_pool(name="ps_acc", bufs=2, space="PSUM") as ps_accp, \
                     tc.tile_pool(name="ps_misc", bufs=1, space="PSUM") as ps_mp:
                    wo_s = wpool3.tile([P, NKB, DM], BF16)
                    for kb in range(NKB):
                        nc.scalar.dma_start(
                            wo_s[:, kb], wo.ap()[kb * P:(kb + 1) * P, :])
                    for b in range(B):
                        for hl in range(HPC):
                            first = (b == 0 and hl == 0)
                            if first:
                                kT_s, v_s = pre_kT, pre_v
                            else:
                                kT_s = pool2.tile([P, T], F32R, tag="kT")
                                v_s = pool2.tile([P, NTB, D], BF16, tag="v")
                                nc.sync.dma_start(
                                    kT_s[:], kT_d[b][hl * P:(hl + 1) * P, :])
                                nc.sync.dma_start(
                                    v_s[:],
                                    v_d[b][:, hl * D:(hl + 1) * D]
                                    .rearrange("(j p) d -> p j d", p=P))
                            for cq in range(NBCH):
                                if first and cq == 0:
                                    qT_s = pre_qT
                                else:
                                    qT_s = pool2.tile([P, TCH], F32R, tag="qT")
                                    nc.sync.dma_start(
                                        qT_s[:],
                                        qT_d[b][hl * P:(hl + 1) * P,
                                                cq * TCH:(cq + 1) * TCH])
                                nblk = 4 * cq + 4
                                ps_ctx = ps_accp.tile([P, TCH], F32, tag="ctx")
                                S = spool.tile([P, TCH], F32R, tag="S")
                                eng = nc.gpsimd if cq in (1, 2) else nc.vector
                                for jp in range(nblk // 2):
                                    j0 = 2 * jp
                                    ps_sc = ps_sp.tile([P, 2, TCH], F32, tag="s")
                                    nc.tensor.matmul(
                                        ps_sc[:, 0], kT_s[:, j0 * P:(j0 + 1) * P],
                                        qT_s[:], start=True, stop=True)
                                    nc.tensor.matmul(
                                        ps_sc[:, 1], kT_s[:, (j0 + 1) * P:(j0 + 2) * P],
                                        qT_s[:], start=True, stop=True)
                                    pT = ppool.tile([P, 2, TCH], BF16, tag="pT")
                                    vmask = j0 - 4 * cq
                                    if vmask >= 0 and not first:
                                        # diagonal pair on warm tiles: skip the
                                        # all-masked column prefix of each block
                                        for h in range(2):
                                            off = (vmask + h) * P
                                            nc.scalar.activation(
                                                pT[:, h, off:], ps_sc[:, h, off:],
                                                EXP, scale=SCALE)
                                    else:
                                        nc.scalar.activation(
                                            pT[:], ps_sc[:], EXP, scale=SCALE)
                                    if vmask >= 0:
                                        nc.vector.tensor_tensor(
                                            pT[:], pT[:],
                                            cm_s[:, vmask * TCH:(vmask + 2) * TCH]
                                            .rearrange("p (v n) -> p v n", v=2), MUL)
                                    # softmax-denominator accumulation offloaded
                                    # from PE to Pool/Vector
                                    if jp == 0:
                                        eng.tensor_tensor(S[:], pT[:, 0], pT[:, 1], ADD)
                                    else:
                                        eng.tensor_tensor(S[:], S[:], pT[:, 0], ADD)
                                        eng.tensor_tensor(S[:], S[:], pT[:, 1], ADD)
                                    for h in range(2):
                                        j = j0 + h
                                        nc.tensor.matmul(
                                            ps_ctx[:], v_s[:, j], pT[:, h],
                                            start=(j == 0), stop=(j == nblk - 1))
                                ps_sum = ps_mp.tile([1, TCH], F32, tag="sum")
                                nc.tensor.matmul(ps_sum[:], onec_s[:], S[:],
                                                 start=True, stop=True)
                                rr = pool2.tile([1, TCH], F32R, tag="rr")
                                with nc.allow_low_precision(reason="plain elementwise recip"):
                                    nc.vector.reciprocal(rr[:], ps_sum[:])
                                ps_bc = ps_mp.tile([P, TCH], F32, tag="bc")
                                nc.tensor.matmul(ps_bc[:], oner_s[:], rr[:],
                                                 start=True, stop=True)
                                bc_s = pool2.tile([P, TCH], F32, tag="bc_s")
                                nc.vector.tensor_copy(bc_s[:], ps_bc[:])
                                ctx_s = pool2.tile([P, TCH], BF16, tag="ctx")
                                nc.vector.tensor_tensor(ctx_s[:], ps_ctx[:], bc_s[:], MUL)
                                # scatter the 512-token chunk into two rank blocks
                                nc.sync.dma_start(
                                    ctxA_d[b][hl]
                                    .rearrange("(r p) n -> p r n", p=P)
                                    [:, 2 * cq:2 * cq + 2],
                                    ctx_s.rearrange("p (r n) -> p r n", r=2))
                            nc.gpsimd.collective_compute(
                                "AllToAll", mybir.AluOpType.bypass,
                                replica_groups=[list(range(NCORES))],
                                ins=[ctxA_d[b][hl][:].opt()],
                                outs=[gout_d[b][hl][:].opt()])

                # ---------------- Phase 3: output projection (full Wo) -----------
                with tc.tile_pool(name="p3", bufs=2) as pool3, \
                     tc.tile_pool(name="ps3", bufs=2, space="PSUM") as ps3:
                    for b in range(B):
                        # g rows must follow wo row-block order: head h = 2r+hl
                        g_t = pool3.tile([P, NTB // 2, HPC, TSL], BF16, tag="g")
                        for hl in range(HPC):
                            nc.scalar.dma_start(
                                g_t[:, :, hl],
                                gout_d[b][hl][:]
                                .rearrange("(r p) n -> p r n", p=P))
                        gv = g_t.rearrange("p r two n -> p (r two) n")
                        for m in range(DM // P):
                            pso = ps3.tile([P, TSL], F32, tag="o")
                            for kb in range(NKB):
                                nc.tensor.matmul(
                                    pso[:], wo_s[:, kb, m * P:(m + 1) * P], gv[:, kb],
                                    start=(kb == 0), stop=(kb == NKB - 1))
                            o_s = pool3.tile([P, TSL], F32, tag="o_s")
                            nc.vector.tensor_copy(o_s[:], pso[:])
                            nc.sync.dma_start(
                                outT.ap()[m * P:(m + 1) * P, b * TSL:(b + 1) * TSL],
                                o_s[:])

    nc.compile()
    return nc


def _prep_inputs(x, cos, sin, Wq, Wk, Wv, Wo):
    x = np.asarray(x, dtype=np.float32)
    cos = np.asarray(cos, dtype=np.float32)
    sin = np.asarray(sin, dtype=np.float32)
    xt = np.ascontiguousarray(x.reshape(BT, DM).T).astype(ml_dtypes.bfloat16)
    cf = np.empty((P, T), np.float32)
    cf[:64] = cos.T
    cf[64:] = cos.T
    sf = np.empty((P, T), np.float32)
    sf[:64] = -sin.T
    sf[64:] = sin.T
    qq = np.arange(TCH, dtype=np.int64)[None, :]
    rr = np.arange(P, dtype=np.int64)[:, None]
    cm = np.concatenate(
        [(qq >= v * P + rr).astype(np.float32) for v in range(TCH // P)],
        axis=1).astype(ml_dtypes.bfloat16)
    onec = np.ones((P, 1), np.float32)
    oner = np.ones((1, P), np.float32)
    wo_full = np.ascontiguousarray(np.asarray(Wo, np.float32)).astype(ml_dtypes.bfloat16)
    in_maps = []
    for c in range(NCORES):
        sl = slice(c * DLOC, (c + 1) * DLOC)
        in_maps.append({
            "xt": xt, "cf": cf, "sf": sf, "cm": cm,
            "onec": onec, "oner": oner,
            "wq": np.ascontiguousarray(np.asarray(Wq, np.float32)[:, sl]).astype(ml_dtypes.bfloat16),
            "wk": np.ascontiguousarray(np.asarray(Wk, np.float32)[:, sl]).astype(ml_dtypes.bfloat16),
            "wv": np.ascontiguousarray(np.asarray(Wv, np.float32)[:, sl]).astype(ml_dtypes.bfloat16),
            "wo": wo_full,
        })
    return in_maps


def run(x, mask, cos, sin, Wq, Wk, Wv, Wo, trace=False):
    global _nc_cache
    if _nc_cache is None:
        _nc_cache = _build()
    in_maps = _prep_inputs(x, cos, sin, Wq, Wk, Wv, Wo)
    res = bass_utils.run_bass_kernel_spmd(
        _nc_cache, in_maps, core_ids=list(range(NCORES)), trace=trace)
    out = np.empty((B, T, DM), np.float32)
    for c in range(NCORES):
        o = res.results[c]["out"]  # [DM, B*TSL]
        for b in range(B):
            out[b, c * TSL:(c + 1) * TSL, :] = o[:, b * TSL:(b + 1) * TSL].T
    return out, res


def kernel(x, mask, cos, sin, Wq, Wk, Wv, Wo):
    out, _ = run(x, mask, cos, sin, Wq, Wk, Wv, Wo, trace=False)
    return out


# revision 6
# speedup vs baseline: 1.1187x; 1.0883x over previous
"""Multi-head causal attention with RoPE on 8 TRN2 NeuronCores.

Tensor-parallel over heads: core c computes heads (2c, 2c+1).
  Phase 1: Q^T,K^T (with RoPE) and V projections from pre-transposed x
           (bf16 inputs/weights, fp32 PSUM accumulate).
  Phase 2: causal attention per (batch, head) in transposed orientation
           (scores^T = K^T_blk^T @ Q^T), softmax without max-subtraction.
           Softmax denominators: exp tiles are accumulated elementwise on
           the Pool+Vector engines (alternating pairs, bf16), then two
           ones-matmuls per 512-query chunk reduce the partial sums over
           the partition axis directly into a broadcast [128,512] PSUM
           tile (no single-partition ops anywhere).  The denominator /
           normalize / scatter stage for chunk n is emitted after chunk
           n+1's score matmuls (one-chunk software pipeline) so the PE
           never waits on the elementwise chain.
  Phase 3: per (batch, head) AllToAll (4 small collectives) redistributes
           context from head-sharded to sequence-sharded while compute
           continues; each core applies the FULL Wo to its 256-token
           slice per batch.
Two DMA trigger queues: the Sync HWDGE queue carries the fine-grained
flow (x chunk 0, RoPE/v writes, ctx scatters, outputs) while the
Activation HWDGE queue carries bulk prefetches (weights, x chunks, Wo
as a single 8MB descriptor, next-section K/V/Q rows, post-collective
gathers) so a waiting bulk load never head-of-line blocks the flow.
"""
import ml_dtypes
import numpy as np

import concourse.bass as bass  # noqa: F401  (engine namespaces live on nc)
import concourse.mybir as mybir
import concourse.tile as tile
from concourse import bacc
from concourse import bass_utils

B, T, DM, H, D = 2, 2048, 2048, 16, 128
NCORES = 8
HPC = H // NCORES        # heads per core
DLOC = HPC * D           # local head width (256)
BT = B * T               # 4096 token rows
P = 128
TCH = 512                # free-dim chunk
NKB = DM // P            # 16 contraction blocks
NTB = T // P             # 16 token blocks per batch
NBCH = T // TCH          # 4 token chunks per batch
TSL = T // NCORES        # 256-token output slice per core per batch
SCALE = 1.0 / float(np.sqrt(D))
F32 = mybir.dt.float32
F32R = mybir.dt.float32r
BF16 = mybir.dt.bfloat16
MUL = mybir.AluOpType.mult
ADD = mybir.AluOpType.add
EXP = mybir.ActivationFunctionType.Exp

_nc_cache = None


def _build():
    nc = bacc.Bacc("TRN2", target_bir_lowering=False, debug=False,
                   num_devices=NCORES)
    xt = nc.dram_tensor("xt", [DM, BT], BF16, kind="ExternalInput")
    wq = nc.dram_tensor("wq", [DM, DLOC], BF16, kind="ExternalInput")
    wk = nc.dram_tensor("wk", [DM, DLOC], BF16, kind="ExternalInput")
    wv = nc.dram_tensor("wv", [DM, DLOC], BF16, kind="ExternalInput")
    wo = nc.dram_tensor("wo", [DM, DM], BF16, kind="ExternalInput")
    cf = nc.dram_tensor("cf", [P, T], F32, kind="ExternalInput")
    sf = nc.dram_tensor("sf", [P, T], F32, kind="ExternalInput")
    cm = nc.dram_tensor("cm", [P, 4 * TCH], BF16, kind="ExternalInput")
    oneb = nc.dram_tensor("oneb", [P, P], BF16, kind="ExternalInput")
    # out^T slice: [out_cols, b0 slice | b1 slice]
    outT = nc.dram_tensor("out", [DM, B * TSL], F32, kind="ExternalOutput")

    with tile.TileContext(nc) as tc:
        with tc.tile_pool(name="dram", bufs=1, space="DRAM") as dpool, \
             tc.tile_pool(name="const", bufs=1) as cpool, \
             tc.tile_pool(name="pre2", bufs=1) as prepool:
            qT_d = [dpool.tile([DLOC, T], F32R, name=f"qT{b}") for b in range(B)]
            kT_d = [dpool.tile([DLOC, T], F32R, name=f"kT{b}") for b in range(B)]
            v_d = [dpool.tile([T, DLOC], BF16, name=f"v{b}") for b in range(B)]
            # A2A per (batch, local head): 8 rank-blocks x [128 hd, 256 t-slice]
            ctxA_d = [[dpool.tile([NCORES * P, TSL], BF16, name=f"ctxA{b}_{hl}")
                       for hl in range(HPC)] for b in range(B)]
            gout_d = [[dpool.tile([NCORES * P, TSL], BF16, name=f"gout{b}_{hl}")
                       for hl in range(HPC)] for b in range(B)]

            cm_s = cpool.tile([P, 4 * TCH], BF16)
            oneb_s = cpool.tile([P, P], BF16)
            pre_kT = prepool.tile([P, T], F32R, name="pre_kT")
            pre_v = prepool.tile([P, NTB, D], BF16, name="pre_v")
            pre_qTF = prepool.tile([P, T], F32R, name="pre_qTF")
            bar_in = dpool.tile([8, 4], F32)
            bar_out = dpool.tile([64, 4], F32, addr_space="Shared")
            # start-skew absorber: cores align here while phase 1 computes
            nc.sync.dma_start(bar_in[:], cf.ap()[0:8, 0:4])
            nc.gpsimd.collective_compute(
                "AllGather", mybir.AluOpType.bypass,
                replica_groups=[list(range(NCORES))],
                ins=[bar_in[:].opt()], outs=[bar_out[:].opt()])

            # ---------------- Phase 1: projections + RoPE ----------------
            with tc.tile_pool(name="p1w", bufs=1) as wpool, \
                 tc.tile_pool(name="p1c", bufs=1) as c1pool, \
                 tc.tile_pool(name="p1", bufs=2) as pool, \
                 tc.tile_pool(name="ps1", bufs=2, space="PSUM") as ps1:
                wq_s = wpool.tile([P, NKB, DLOC], BF16)
                wk_s = wpool.tile([P, NKB, DLOC], BF16)
                wv_s = wpool.tile([P, NKB, DLOC], BF16)
                cf_s = c1pool.tile([P, T], F32)
                sf_s = c1pool.tile([P, T], F32)
                # critical path: wq + x chunk 0 on the sync queue
                nc.sync.dma_start(wq_s[:], wq.ap().rearrange("(kb p) m -> p kb m", p=P))

                XCH = TCH
                for i in range(BT // XCH):
                    bb, ic = i // (T // XCH), i % (T // XCH)
                    xt_t = pool.tile([P, NKB, XCH], BF16, tag="xt")
                    (nc.sync if i == 0 else nc.scalar).dma_start(
                        xt_t[:],
                        xt.ap()[:, i * XCH:(i + 1) * XCH]
                        .rearrange("(kb p) n -> p kb n", p=P))
                    if i == 0:
                        nc.sync.dma_start(cf_s[:], cf.ap())
                        nc.sync.dma_start(sf_s[:], sf.ap())
                        nc.scalar.dma_start(
                            wk_s[:], wk.ap().rearrange("(kb p) m -> p kb m", p=P))
                        nc.scalar.dma_start(
                            wv_s[:], wv.ap().rearrange("(kb p) m -> p kb m", p=P))
                    cs = cf_s[:, ic * XCH:(ic + 1) * XCH]
                    sn = sf_s[:, ic * XCH:(ic + 1) * XCH]
                    for w_s, dst in ((wq_s, qT_d), (wk_s, kT_d)):
                        for m in range(HPC):
                            ps = ps1.tile([P, XCH], F32, tag="qk")
                            for kb in range(NKB):
                                nc.tensor.matmul(
                                    ps[:], w_s[:, kb, m * P:(m + 1) * P],
                                    xt_t[:, kb],
                                    start=(kb == 0), stop=(kb == NKB - 1))
                            # RoPE: rq = q*cos_full + rot(q)*sin_signed
                            tmp = pool.tile([P, XCH], F32, tag="tmp")
                            tmp2 = pool.tile([P, XCH], F32, tag="tmp2")
                            rq = pool.tile([P, XCH], F32R, tag="rq")
                            nc.vector.tensor_tensor(tmp[0:64], ps[64:128], sn[0:64], MUL)
                            nc.vector.tensor_tensor(tmp[64:128], ps[0:64], sn[64:128], MUL)
                            nc.vector.tensor_tensor(tmp2[:], ps[:], cs, MUL)
                            nc.vector.tensor_tensor(rq[:], tmp2[:], tmp[:], ADD)
                            nc.sync.dma_start(
                                dst[bb][m * P:(m + 1) * P, ic * XCH:(ic + 1) * XCH],
                                rq[:])
                    for tb in range(XCH // P):
                        psv = ps1.tile([P, DLOC], F32, tag="v")
                        for kb in range(NKB):
                            nc.tensor.matmul(
                                psv[:], xt_t[:, kb, tb * P:(tb + 1) * P],
                                wv_s[:, kb],
                                start=(kb == 0), stop=(kb == NKB - 1))
                        vsb = pool.tile([P, DLOC], BF16, tag="vsb")
                        nc.vector.tensor_copy(vsb[:], psv[:])
                        r0 = ic * XCH + tb * P
                        nc.sync.dma_start(v_d[bb][r0:r0 + P, :], vsb[:])

                # phase-2 constants + attention section-1 inputs, bulk queue
                nc.scalar.dma_start(cm_s[:], cm.ap())
                nc.scalar.dma_start(oneb_s[:], oneb.ap())
                nc.scalar.dma_start(pre_kT[:], kT_d[0][0:P, :])
                nc.scalar.dma_start(
                    pre_v[:],
                    v_d[0][:, 0:D].rearrange("(j p) d -> p j d", p=P))
                nc.scalar.dma_start(pre_qTF[:], qT_d[0][0:P, :])

            # wo_s pool opens after phase-1 pools close, so its 8MB
            # reuses phase-1 SBUF and the load runs during attention.
            with tc.tile_pool(name="p3w", bufs=1) as wpool3:
                # ------------- Phase 2: causal attention + split A2A -------------
                with tc.tile_pool(name="p2", bufs=3) as pool2, \
                     tc.tile_pool(name="p2s", bufs=2) as spool, \
                     tc.tile_pool(name="p2t", bufs=6) as ppool, \
                     tc.tile_pool(name="ps_s", bufs=2, space="PSUM") as ps_sp, \
                     tc.tile_pool(name="ps_acc", bufs=2, space="PSUM") as ps_accp, \
                     tc.tile_pool(name="ps_bcp", bufs=2, space="PSUM") as ps_bcp:
                    wo_s = wpool3.tile([P, NKB, DM], BF16)
                    nc.scalar.dma_start(
                        wo_s[:], wo.ap().rearrange("(kb p) m -> p kb m", p=P))

                    secs = [(b, hl) for b in range(B) for hl in range(HPC)]
                    cur = (pre_kT, pre_v, pre_qTF)
                    nxt = None

                    def finalize(pend):
                        cq, S_v, S_p, ps_ctx, bq, hq = pend
                        ps_bc = ps_bcp.tile([P, TCH], F32, tag="bc")
                        nc.tensor.matmul(ps_bc[:], oneb_s[:], S_v[:, 0],
                                         start=True, stop=False)
                        nc.tensor.matmul(ps_bc[:], oneb_s[:], S_p[:, 0],
                                         start=False, stop=True)
                        bc_s = pool2.tile([P, TCH], F32, tag="bc_s")
                        with nc.allow_low_precision(reason="plain elementwise recip"):
                            nc.vector.reciprocal(bc_s[:], ps_bc[:])
                        ctx_s = pool2.tile([P, TCH], BF16, tag="ctx")
                        nc.vector.tensor_tensor(ctx_s[:], ps_ctx[:], bc_s[:], MUL)
                        nc.sync.dma_start(
                            ctxA_d[bq][hq]
                            .rearrange("(r p) n -> p r n", p=P)
                            [:, 2 * cq:2 * cq + 2],
                            ctx_s.rearrange("p (r n) -> p r n", r=2))

                    for s, (b, hl) in enumerate(secs):
                        kT_s, v_s, qTF_s = cur
                        first = (s == 0)
                        pend = None
                        for cq in range(NBCH):
                            if cq == 1 and s + 1 < len(secs):
                                b2, h2 = secs[s + 1]
                                nxt = (pool2.tile([P, T], F32R, tag="kT", name="kT_n"),
                                       pool2.tile([P, NTB, D], BF16, tag="v", name="v_n"),
                                       pool2.tile([P, T], F32R, tag="qTF", name="qTF_n"))
                                nc.scalar.dma_start(
                                    nxt[0][:], kT_d[b2][h2 * P:(h2 + 1) * P, :])
                                nc.scalar.dma_start(
                                    nxt[1][:],
                                    v_d[b2][:, h2 * D:(h2 + 1) * D]
                                    .rearrange("(j p) d -> p j d", p=P))
                                nc.scalar.dma_start(
                                    nxt[2][:], qT_d[b2][h2 * P:(h2 + 1) * P, :])
                            nblk = 4 * cq + 4
                            qT_c = qTF_s[:, cq * TCH:(cq + 1) * TCH]
                            ps_ctx = ps_accp.tile([P, TCH], F32, tag="ctx")
                            S_v = spool.tile([P, 2, TCH], BF16, tag="Sv")
                            S_p = spool.tile([P, 2, TCH], BF16, tag="Sp")
                            seen = {id(S_v): False, id(S_p): False}
                            for jp in range(nblk // 2):
                                j0 = 2 * jp
                                ps_sc = ps_sp.tile([P, 2, TCH], F32, tag="s")
                                nc.tensor.matmul(
                                    ps_sc[:, 0], kT_s[:, j0 * P:(j0 + 1) * P],
                                    qT_c, start=True, stop=True)
                                nc.tensor.matmul(
                                    ps_sc[:, 1], kT_s[:, (j0 + 1) * P:(j0 + 2) * P],
                                    qT_c, start=True, stop=True)
                                pT = ppool.tile([P, 2, TCH], BF16, tag="pT")
                                vmask = j0 - 4 * cq
                                if vmask >= 0 and not first:
                                    # diagonal pair on warm tiles: skip the
                                    # all-masked column prefix of each block
                                    for h in range(2):
                                        off = (vmask + h) * P
                                        nc.scalar.activation(
                                            pT[:, h, off:], ps_sc[:, h, off:],
                                            EXP, scale=SCALE)
                                else:
                                    nc.scalar.activation(
                                        pT[:], ps_sc[:], EXP, scale=SCALE)
                                if vmask >= 0:
                                    # full-width: ctx matmuls read all columns,
                                    # so every masked entry must be zeroed
                                    nc.vector.tensor_tensor(
                                        pT[:], pT[:],
                                        cm_s[:, vmask * TCH:(vmask + 2) * TCH]
                                        .rearrange("p (v n) -> p v n", v=2), MUL)
                                # softmax-denominator partials on Pool/Vector
                                eng = nc.vector if jp % 2 == 0 else nc.gpsimd
                                S_e = S_v if jp % 2 == 0 else S_p
                                if not seen[id(S_e)]:
                                    eng.tensor_copy(S_e[:], pT[:])
                                    seen[id(S_e)] = True
                                elif vmask >= 0:
                                    c0 = vmask * P
                                    eng.tensor_tensor(
                                        S_e[:, :, c0:], S_e[:, :, c0:],
                                        pT[:, :, c0:], ADD)
                                else:
                                    eng.tensor_tensor(S_e[:], S_e[:], pT[:], ADD)
                                for h in range(2):
                                    j = j0 + h
                                    nc.tensor.matmul(
                                        ps_ctx[:], v_s[:, j], pT[:, h],
                                        start=(j == 0), stop=(j == nblk - 1))
                            # fold each engine's two halves
                            nc.vector.tensor_tensor(S_v[:, 0], S_v[:, 0], S_v[:, 1], ADD)
                            nc.gpsimd.tensor_tensor(S_p[:, 0], S_p[:, 0], S_p[:, 1], ADD)
                            if pend is not None:
                                finalize(pend)
                            pend = (cq, S_v, S_p, ps_ctx, b, hl)
                        finalize(pend)
                        nc.gpsimd.collective_compute(
                            "AllToAll", mybir.AluOpType.bypass,
                            replica_groups=[list(range(NCORES))],
                            ins=[ctxA_d[b][hl][:].opt()],
                            outs=[gout_d[b][hl][:].opt()])
                        cur = nxt

                # ---------------- Phase 3: output projection (full Wo) -----------
                with tc.tile_pool(name="p3", bufs=2) as pool3, \
                     tc.tile_pool(name="ps3", bufs=2, space="PSUM") as ps3:
                    for b in range(B):
                        # g rows must follow wo row-block order: head h = 2r+hl
                        g_t = pool3.tile([P, NTB // 2, HPC, TSL], BF16, tag="g")
                        for hl in range(HPC):
                            nc.scalar.dma_start(
                                g_t[:, :, hl],
                                gout_d[b][hl][:]
                                .rearrange("(r p) n -> p r n", p=P))
                        gv = g_t.rearrange("p r two n -> p (r two) n")
                        for m in range(DM // P):
                            pso = ps3.tile([P, TSL], F32, tag="o")
                            for kb in range(NKB):
                                nc.tensor.matmul(
                                    pso[:], wo_s[:, kb, m * P:(m + 1) * P], gv[:, kb],
                                    start=(kb == 0), stop=(kb == NKB - 1))
                            o_s = pool3.tile([P, TSL], F32, tag="o_s")
                            nc.vector.tensor_copy(o_s[:], pso[:])
                            nc.sync.dma_start(
                                outT.ap()[m * P:(m + 1) * P, b * TSL:(b + 1) * TSL],
                                o_s[:])

    nc.compile()
    return nc


def _prep_inputs(x, cos, sin, Wq, Wk, Wv, Wo):
    x = np.asarray(x, dtype=np.float32)
    cos = np.asarray(cos, dtype=np.float32)
    sin = np.asarray(sin, dtype=np.float32)
    xt = np.ascontiguousarray(x.reshape(BT, DM).T).astype(ml_dtypes.bfloat16)
    cf = np.empty((P, T), np.float32)
    cf[:64] = cos.T
    cf[64:] = cos.T
    sf = np.empty((P, T), np.float32)
    sf[:64] = -sin.T
    sf[64:] = sin.T
    qq = np.arange(TCH, dtype=np.int64)[None, :]
    rr = np.arange(P, dtype=np.int64)[:, None]
    cm = np.concatenate(
        [(qq >= v * P + rr).astype(np.float32) for v in range(TCH // P)],
        axis=1).astype(ml_dtypes.bfloat16)
    oneb = np.ones((P, P), np.float32).astype(ml_dtypes.bfloat16)
    wo_full = np.ascontiguousarray(np.asarray(Wo, np.float32)).astype(ml_dtypes.bfloat16)
    in_maps = []
    for c in range(NCORES):
        sl = slice(c * DLOC, (c + 1) * DLOC)
        in_maps.append({
            "xt": xt, "cf": cf, "sf": sf, "cm": cm, "oneb": oneb,
            "wq": np.ascontiguousarray(np.asarray(Wq, np.float32)[:, sl]).astype(ml_dtypes.bfloat16),
            "wk": np.ascontiguousarray(np.asarray(Wk, np.float32)[:, sl]).astype(ml_dtypes.bfloat16),
            "wv": np.ascontiguousarray(np.asarray(Wv, np.float32)[:, sl]).astype(ml_dtypes.bfloat16),
            "wo": wo_full,
        })
    return in_maps


def run(x, mask, cos, sin, Wq, Wk, Wv, Wo, trace=False):
    global _nc_cache
    if _nc_cache is None:
        _nc_cache = _build()
    in_maps = _prep_inputs(x, cos, sin, Wq, Wk, Wv, Wo)
    res = bass_utils.run_bass_kernel_spmd(
        _nc_cache, in_maps, core_ids=list(range(NCORES)), trace=trace)
    out = np.empty((B, T, DM), np.float32)
    for c in range(NCORES):
        o = res.results[c]["out"]  # [DM, B*TSL]
        for b in range(B):
            out[b, c * TSL:(c + 1) * TSL, :] = o[:, b * TSL:(b + 1) * TSL].T
    return out, res


def kernel(x, mask, cos, sin, Wq, Wk, Wv, Wo):
    out, _ = run(x, mask, cos, sin, Wq, Wk, Wv, Wo, trace=False)
    return out


# revision 9
# speedup vs baseline: 1.1773x; 1.0524x over previous
"""Multi-head causal attention with RoPE on 8 TRN2 NeuronCores.

Tensor-parallel over heads: core c computes heads (2c, 2c+1).
  Phase 1: Q^T,K^T (with RoPE) and V projections from host-packed bf16
           x/weights.  Q^T/K^T (bf16, post-RoPE) and V (bf16) are written
           DIRECTLY into persistent SBUF tiles — no DRAM roundtrip, so
           phase 2 needs no loads at all.
  Phase 2: causal attention per (batch, head), scores^T = K^T_blk^T @ Q^T,
           softmax without max-subtraction.  Softmax denominators: exp
           tiles are accumulated elementwise (bf16) on Vector (Pool for
           the middle chunks), then two ones-matmuls per 512-query chunk
           reduce the halves over the partition axis directly into a
           broadcast [128,512] PSUM tile.  The denominator / normalize /
           scatter stage of chunk n is emitted after chunk n+1's compute
           (one-chunk software pipeline) so the PE never waits on it.
  Phase 3: per (batch, head) AllToAll (4 small collectives, all hidden
           under compute) redistributes context from head-sharded to
           sequence-sharded; each core applies the FULL Wo to its
           256-token slice per batch.
All bulk inputs are host-packed so each DMA is 128 large descriptors
(DMA trigger time is proportional to descriptor count and blocks the
issuing engine's sequencer).  Sync HWDGE queue: weights/consts, ctx
scatters, gathers, outputs.  Activation HWDGE queue: x chunks.
"""
import ml_dtypes
import numpy as np

import concourse.bass as bass  # noqa: F401  (engine namespaces live on nc)
import concourse.mybir as mybir
import concourse.tile as tile
from concourse import bacc
from concourse import bass_utils

B, T, DM, H, D = 2, 2048, 2048, 16, 128
NCORES = 8
HPC = H // NCORES        # heads per core
DLOC = HPC * D           # local head width (256)
BT = B * T               # 4096 token rows
P = 128
TCH = 512                # free-dim chunk
XCH = 512                # phase-1 token chunk
NKB = DM // P            # 16 contraction blocks
NTB = T // P             # 16 token blocks per batch
NBCH = T // TCH          # 4 token chunks per batch
TSL = T // NCORES        # 256-token output slice per core per batch
SCALE = 1.0 / float(np.sqrt(D))
F32 = mybir.dt.float32
BF16 = mybir.dt.bfloat16
MUL = mybir.AluOpType.mult
ADD = mybir.AluOpType.add
EXP = mybir.ActivationFunctionType.Exp

_nc_cache = None


def _build():
    nc = bacc.Bacc("TRN2", target_bir_lowering=False, debug=False,
                   num_devices=NCORES)
    xt = nc.dram_tensor("xt", [BT // XCH, P, NKB * XCH], BF16, kind="ExternalInput")
    wq = nc.dram_tensor("wq", [P, NKB * DLOC], BF16, kind="ExternalInput")
    wk = nc.dram_tensor("wk", [P, NKB * DLOC], BF16, kind="ExternalInput")
    wv = nc.dram_tensor("wv", [P, NKB * DLOC], BF16, kind="ExternalInput")
    wo = nc.dram_tensor("wo", [P, NKB * DM], BF16, kind="ExternalInput")
    cf = nc.dram_tensor("cf", [P, T], F32, kind="ExternalInput")
    sf = nc.dram_tensor("sf", [P, T], F32, kind="ExternalInput")
    cm = nc.dram_tensor("cm", [P, 4 * TCH], BF16, kind="ExternalInput")
    oneb = nc.dram_tensor("oneb", [P, P], BF16, kind="ExternalInput")
    # out^T slice: [out_cols, b0 slice | b1 slice]
    outT = nc.dram_tensor("out", [DM, B * TSL], F32, kind="ExternalOutput")

    with tile.TileContext(nc) as tc:
        with tc.tile_pool(name="dram", bufs=1, space="DRAM") as dpool, \
             tc.tile_pool(name="persist", bufs=1) as keep:
            # A2A per (batch, local head): 8 rank-blocks x [128 hd, 256 t-slice]
            ctxA_d = [[dpool.tile([NCORES * P, TSL], BF16, name=f"ctxA{b}_{hl}")
                       for hl in range(HPC)] for b in range(B)]
            gout_d = [[dpool.tile([NCORES * P, TSL], BF16, name=f"gout{b}_{hl}")
                       for hl in range(HPC)] for b in range(B)]

            # persistent SBUF: q/k (bf16, post-RoPE, [dims, keys]) and v
            # ([keys, j, d]) for every (batch, local-head) section
            qT_a = [[keep.tile([P, T], BF16, name=f"qTa{b}_{hl}")
                     for hl in range(HPC)] for b in range(B)]
            kT_a = [[keep.tile([P, T], BF16, name=f"kTa{b}_{hl}")
                     for hl in range(HPC)] for b in range(B)]
            v_a = [[keep.tile([P, NTB, D], BF16, name=f"va{b}_{hl}")
                    for hl in range(HPC)] for b in range(B)]
            cm_s = keep.tile([P, 4 * TCH], BF16)
            oneb_s = keep.tile([P, P], BF16)
            bar_in = dpool.tile([8, 4], F32)
            bar_out = dpool.tile([64, 4], F32, addr_space="Shared")
            # start-skew absorber: cores align here while phase 1 computes
            nc.sync.dma_start(bar_in[:], cf.ap()[0:8, 0:4])
            nc.gpsimd.collective_compute(
                "AllGather", mybir.AluOpType.bypass,
                replica_groups=[list(range(NCORES))],
                ins=[bar_in[:].opt()], outs=[bar_out[:].opt()])


            # ---------------- Phase 1: projections + RoPE ----------------
            with tc.tile_pool(name="p1w", bufs=1) as wpool, \
                 tc.tile_pool(name="p1", bufs=2) as pool, \
                 tc.tile_pool(name="ps1", bufs=2, space="PSUM") as ps1:
                wq_s = wpool.tile([P, NKB, DLOC], BF16)
                wk_s = wpool.tile([P, NKB, DLOC], BF16)
                wv_s = wpool.tile([P, NKB, DLOC], BF16)
                cf_s = wpool.tile([P, T], F32)
                sf_s = wpool.tile([P, T], F32)
                nc.sync.dma_start(wq_s[:], wq.ap().rearrange("p (kb m) -> p kb m", kb=NKB))

                for i in range(BT // XCH):
                    bb, ic = i // (T // XCH), i % (T // XCH)
                    xt_t = pool.tile([P, NKB, XCH], BF16, tag="xt")
                    nc.scalar.dma_start(
                        xt_t[:], xt.ap()[i].rearrange("p (kb n) -> p kb n", kb=NKB))
                    if i == 0:
                        nc.sync.dma_start(cf_s[:], cf.ap())
                        nc.sync.dma_start(sf_s[:], sf.ap())
                        nc.scalar.dma_start(
                            wk_s[:], wk.ap().rearrange("p (kb m) -> p kb m", kb=NKB))
                        nc.scalar.dma_start(
                            wv_s[:], wv.ap().rearrange("p (kb m) -> p kb m", kb=NKB))
                        nc.sync.dma_start(cm_s[:], cm.ap())
                        nc.sync.dma_start(oneb_s[:], oneb.ap())
                    cs = cf_s[:, ic * XCH:(ic + 1) * XCH]
                    sn = sf_s[:, ic * XCH:(ic + 1) * XCH]
                    for w_s, dst in ((wq_s, qT_a), (wk_s, kT_a)):
                        for m in range(HPC):
                            ps = ps1.tile([P, XCH], F32, tag="qk")
                            for kb in range(NKB):
                                nc.tensor.matmul(
                                    ps[:], w_s[:, kb, m * P:(m + 1) * P],
                                    xt_t[:, kb],
                                    start=(kb == 0), stop=(kb == NKB - 1))
                            # RoPE: rq = q*cos_full + rot(q)*sin_signed,
                            # written straight into the persistent bf16 tile
                            tmp = pool.tile([P, XCH], F32, tag="tmp")
                            tmp2 = pool.tile([P, XCH], F32, tag="tmp2")
                            nc.vector.tensor_tensor(tmp[0:64], ps[64:128], sn[0:64], MUL)
                            nc.vector.tensor_tensor(tmp[64:128], ps[0:64], sn[64:128], MUL)
                            nc.vector.tensor_tensor(tmp2[:], ps[:], cs, MUL)
                            nc.vector.tensor_tensor(
                                dst[bb][m][:, ic * XCH:(ic + 1) * XCH],
                                tmp2[:], tmp[:], ADD)
                    for tb in range(XCH // P):
                        psv = ps1.tile([P, DLOC], F32, tag="v")
                        for kb in range(NKB):
                            nc.tensor.matmul(
                                psv[:], xt_t[:, kb, tb * P:(tb + 1) * P],
                                wv_s[:, kb],
                                start=(kb == 0), stop=(kb == NKB - 1))
                        jx = ic * (XCH // P) + tb
                        nc.vector.tensor_copy(v_a[bb][0][:, jx], psv[:, 0:D])
                        nc.vector.tensor_copy(v_a[bb][1][:, jx], psv[:, D:DLOC])

            # ------------- Phase 2: causal attention + split A2A -------------
            with tc.tile_pool(name="p3w", bufs=1) as wpool3:
                g_ts = [wpool3.tile([P, HPC, NCORES, TSL], BF16, name=f"g{b}_t")
                        for b in range(B)]
                with tc.tile_pool(name="p2", bufs=3) as pool2, \
                     tc.tile_pool(name="p2s", bufs=2) as spool, \
                     tc.tile_pool(name="p2t", bufs=6) as ppool, \
                     tc.tile_pool(name="ps_s", bufs=2, space="PSUM") as ps_sp, \
                     tc.tile_pool(name="ps_acc", bufs=2, space="PSUM") as ps_accp, \
                     tc.tile_pool(name="ps_bcp", bufs=2, space="PSUM") as ps_bcp:
                    wo_s = wpool3.tile([P, NKB, DM], BF16)
                    nc.sync.dma_start(
                        wo_s[:], wo.ap().rearrange("p (kb m) -> p kb m", kb=NKB))

                    secs = [(b, hl) for b in range(B) for hl in range(HPC)]

                    def finalize(pend):
                        cq, S, ps_ctx, bq, hq = pend
                        ps_bc = ps_bcp.tile([P, TCH], F32, tag="bc")
                        nc.tensor.matmul(ps_bc[:], oneb_s[:], S[:, 0],
                                         start=True, stop=False)
                        nc.tensor.matmul(ps_bc[:], oneb_s[:], S[:, 1],
                                         start=False, stop=True)
                        bc_s = pool2.tile([P, TCH], F32, tag="bc_s")
                        with nc.allow_low_precision(reason="plain elementwise recip"):
                            nc.vector.reciprocal(bc_s[:], ps_bc[:])
                        ctx_s = pool2.tile([P, TCH], BF16, tag="ctx")
                        nc.vector.tensor_tensor(ctx_s[:], ps_ctx[:], bc_s[:], MUL)
                        nc.sync.dma_start(
                            ctxA_d[bq][hq]
                            .rearrange("(r p) n -> p r n", p=P)
                            [:, 2 * cq:2 * cq + 2],
                            ctx_s.rearrange("p (r n) -> p r n", r=2))

                    for s, (b, hl) in enumerate(secs):
                        kT_s, v_s, qTF_s = kT_a[b][hl], v_a[b][hl], qT_a[b][hl]
                        first = (s == 0)
                        if s == 3:
                            # prefetch batch-0 gather during the last section
                            # (A2As for batch 0 finished during section 3)
                            for h2 in range(HPC):
                                nc.sync.dma_start(
                                    g_ts[0][:, h2],
                                    gout_d[0][h2][:]
                                    .rearrange("(r p) n -> p r n", p=P))
                        pend = None
                        for cq in range(NBCH):
                            nblk = 4 * cq + 4
                            qT_c = qTF_s[:, cq * TCH:(cq + 1) * TCH]
                            ps_ctx = ps_accp.tile([P, TCH], F32, tag="ctx")
                            S = spool.tile([P, 2, TCH], BF16, tag="S")
                            eng = nc.gpsimd if cq in (1, 2) else nc.vector
                            s_tiles = 0
                            pT0 = None
                            for jp in range(nblk // 2):
                                j0 = 2 * jp
                                ps_sc = ps_sp.tile([P, 2, TCH], F32, tag="s")
                                nc.tensor.matmul(
                                    ps_sc[:, 0], kT_s[:, j0 * P:(j0 + 1) * P],
                                    qT_c, start=True, stop=True)
                                nc.tensor.matmul(
                                    ps_sc[:, 1], kT_s[:, (j0 + 1) * P:(j0 + 2) * P],
                                    qT_c, start=True, stop=True)
                                pT = ppool.tile([P, 2, TCH], BF16, tag="pT")
                                vmask = j0 - 4 * cq
                                if vmask >= 0 and not first:
                                    # diagonal pair on warm tiles: skip the
                                    # all-masked column prefix of each block
                                    for h in range(2):
                                        off = (vmask + h) * P
                                        nc.scalar.activation(
                                            pT[:, h, off:], ps_sc[:, h, off:],
                                            EXP, scale=SCALE)
                                else:
                                    nc.scalar.activation(
                                        pT[:], ps_sc[:], EXP, scale=SCALE)
                                if vmask >= 0:
                                    # full-width: ctx matmuls read all columns,
                                    # so every masked entry must be zeroed
                                    nc.vector.tensor_tensor(
                                        pT[:], pT[:],
                                        cm_s[:, vmask * TCH:(vmask + 2) * TCH]
                                        .rearrange("p (v n) -> p v n", v=2), MUL)
                                # softmax-denominator partials, off the PE
                                s_tiles += 1
                                if s_tiles == 1:
                                    pT0 = pT
                                elif s_tiles == 2:
                                    eng.tensor_tensor(S[:], pT0[:], pT[:], ADD)
                                elif vmask >= 0:
                                    c0 = vmask * P
                                    eng.tensor_tensor(
                                        S[:, :, c0:], S[:, :, c0:],
                                        pT[:, :, c0:], ADD)
                                else:
                                    eng.tensor_tensor(S[:], S[:], pT[:], ADD)
                                for h in range(2):
                                    j = j0 + h
                                    nc.tensor.matmul(
                                        ps_ctx[:], v_s[:, j], pT[:, h],
                                        start=(j == 0), stop=(j == nblk - 1))
                            if pend is not None:
                                finalize(pend)
                            pend = (cq, S, ps_ctx, b, hl)
                        finalize(pend)
                        nc.gpsimd.collective_compute(
                            "AllToAll", mybir.AluOpType.bypass,
                            replica_groups=[list(range(NCORES))],
                            ins=[ctxA_d[b][hl][:].opt()],
                            outs=[gout_d[b][hl][:].opt()])

                # ---------------- Phase 3: output projection (full Wo) -----------
                with tc.tile_pool(name="p3", bufs=2) as pool3, \
                     tc.tile_pool(name="ps3", bufs=2, space="PSUM") as ps3:
                    for b in range(B):
                        g_t = g_ts[b]
                        if b == 1:
                            for h2 in range(HPC):
                                nc.sync.dma_start(
                                    g_t[:, h2],
                                    gout_d[1][h2][:]
                                    .rearrange("(r p) n -> p r n", p=P))
                        for m in range(DM // P):
                            pso = ps3.tile([P, TSL], F32, tag="o")
                            for kb in range(NKB):
                                nc.tensor.matmul(
                                    pso[:], wo_s[:, kb, m * P:(m + 1) * P],
                                    g_t[:, kb % 2, kb // 2],
                                    start=(kb == 0), stop=(kb == NKB - 1))
                            o_s = pool3.tile([P, TSL], F32, tag="o_s")
                            nc.vector.tensor_copy(o_s[:], pso[:])
                            nc.sync.dma_start(
                                outT.ap()[m * P:(m + 1) * P, b * TSL:(b + 1) * TSL],
                                o_s[:])

    nc.compile()
    return nc


def _prep_inputs(x, cos, sin, Wq, Wk, Wv, Wo):
    x = np.asarray(x, dtype=np.float32)
    cos = np.asarray(cos, dtype=np.float32)
    sin = np.asarray(sin, dtype=np.float32)
    # xt packed: [chunk, partition, kb*XCH] so each chunk load is 128
    # contiguous 16KB descriptors
    xt2 = np.ascontiguousarray(x.reshape(BT, DM).T)          # [DM, BT]
    xtp = np.ascontiguousarray(
        xt2.reshape(NKB, P, BT // XCH, XCH).transpose(2, 1, 0, 3)
        .reshape(BT // XCH, P, NKB * XCH)).astype(ml_dtypes.bfloat16)
    cf = np.empty((P, T), np.float32)
    cf[:64] = cos.T
    cf[64:] = cos.T
    sf = np.empty((P, T), np.float32)
    sf[:64] = -sin.T
    sf[64:] = sin.T
    qq = np.arange(TCH, dtype=np.int64)[None, :]
    rr = np.arange(P, dtype=np.int64)[:, None]
    cm = np.concatenate(
        [(qq >= v * P + rr).astype(np.float32) for v in range(TCH // P)],
        axis=1).astype(ml_dtypes.bfloat16)
    oneb = np.ones((P, P), np.float32).astype(ml_dtypes.bfloat16)

    def pack_w(w):  # [DM, M] -> [P, NKB*M]
        w = np.asarray(w, np.float32)
        m = w.shape[1]
        return np.ascontiguousarray(
            w.reshape(NKB, P, m).transpose(1, 0, 2).reshape(P, NKB * m)
        ).astype(ml_dtypes.bfloat16)

    wo_p = pack_w(Wo)
    in_maps = []
    for c in range(NCORES):
        sl = slice(c * DLOC, (c + 1) * DLOC)
        in_maps.append({
            "xt": xtp, "cf": cf, "sf": sf, "cm": cm, "oneb": oneb,
            "wq": pack_w(np.asarray(Wq, np.float32)[:, sl]),
            "wk": pack_w(np.asarray(Wk, np.float32)[:, sl]),
            "wv": pack_w(np.asarray(Wv, np.float32)[:, sl]),
            "wo": wo_p,
        })
    return in_maps


def run(x, mask, cos, sin, Wq, Wk, Wv, Wo, trace=False):
    global _nc_cache
    if _nc_cache is None:
        _nc_cache = _build()
    in_maps = _prep_inputs(x, cos, sin, Wq, Wk, Wv, Wo)
    res = bass_utils.run_bass_kernel_spmd(
        _nc_cache, in_maps, core_ids=list(range(NCORES)), trace=trace)
    out = np.empty((B, T, DM), np.float32)
    for c in range(NCORES):
        o = res.results[c]["out"]  # [DM, B*TSL]
        for b in range(B):
            out[b, c * TSL:(c + 1) * TSL, :] = o[:, b * TSL:(b + 1) * TSL].T
    return out, res


def kernel(x, mask, cos, sin, Wq, Wk, Wv, Wo):
    out, _ = run(x, mask, cos, sin, Wq, Wk, Wv, Wo, trace=False)
    return out


# revision 12
# speedup vs baseline: 1.3384x; 1.1368x over previous
"""Multi-head causal attention with RoPE on 8 TRN2 NeuronCores.

Tensor-parallel over heads: core c computes heads (2c, 2c+1).
  Phase 1: Q^T,K^T (with RoPE) and V projections from host-packed bf16
           x/weights.  Q^T/K^T (bf16, post-RoPE) and V (bf16) are written
           DIRECTLY into persistent SBUF tiles — no DRAM roundtrip, so
           phase 2 needs no loads at all.
  Phase 2: causal attention per (batch, head), scores^T = K^T_blk^T @ Q^T,
           softmax without max-subtraction.  Softmax denominators: exp
           tiles are accumulated elementwise (bf16) on the Vector engine,
           then two ones-matmuls per 512-query chunk reduce the halves
           over the partition axis directly into a broadcast [128,512]
           PSUM tile; 1/x uses the fast custom-DVE approximation (plain
           InstReciprocal costs ~4us fixed).  The denominator / normalize
           / scatter stage of chunk n is emitted inside chunk n+1's first
           key-pair (one-chunk software pipeline) so the PE and the ctx
           PSUM pool never wait on it.
  Phase 3: per-batch AllToAll (2 collectives; the batch-0 one hides under
           batch-1 attention, the batch-1 one under batch-0's Wo)
           redistributes context from head-sharded to sequence-sharded;
           each core applies the FULL Wo to its 256-token slice per
           batch.  Wo is host-packed in gather-slot order (row u=16p+t ->
           [p,t]) so the post-collective gather is a single linear DMA
           (128 contiguous 8KB descriptors) instead of 2048 512B ones.
All bulk inputs are host-packed so each DMA is 128 large descriptors
(DMA trigger time is proportional to descriptor count and blocks the
issuing engine's sequencer).  Sync HWDGE queue: weights/consts, ctx
scatters, gathers, outputs.  Activation HWDGE queue: x chunks.
"""
import ml_dtypes
import numpy as np

import concourse.bass as bass  # noqa: F401  (engine namespaces live on nc)
import concourse.mybir as mybir
import concourse.tile as tile
from concourse import bacc
from concourse import bass_utils

B, T, DM, H, D = 2, 2048, 2048, 16, 128
NCORES = 8
HPC = H // NCORES        # heads per core
DLOC = HPC * D           # local head width (256)
BT = B * T               # 4096 token rows
P = 128
TCH = 512                # free-dim chunk
XCH = 512                # phase-1 token chunk
NKB = DM // P            # 16 contraction blocks
NTB = T // P             # 16 token blocks per batch
NBCH = T // TCH          # 4 token chunks per batch
TSL = T // NCORES        # 256-token output slice per core per batch
SCALE = 1.0 / float(np.sqrt(D))
F32 = mybir.dt.float32
BF16 = mybir.dt.bfloat16
MUL = mybir.AluOpType.mult
ADD = mybir.AluOpType.add
EXP = mybir.ActivationFunctionType.Exp

_nc_cache = None


def _build():
    nc = bacc.Bacc("TRN2", target_bir_lowering=False, debug=False,
                   num_devices=NCORES)
    xt = nc.dram_tensor("xt", [BT // XCH, P, NKB * XCH], BF16, kind="ExternalInput")
    wq = nc.dram_tensor("wq", [P, NKB * DLOC], BF16, kind="ExternalInput")
    wk = nc.dram_tensor("wk", [P, NKB * DLOC], BF16, kind="ExternalInput")
    wv = nc.dram_tensor("wv", [P, NKB * DLOC], BF16, kind="ExternalInput")
    wo = nc.dram_tensor("wo", [P, NKB * DM], BF16, kind="ExternalInput")
    cf = nc.dram_tensor("cf", [P, T], F32, kind="ExternalInput")
    sf = nc.dram_tensor("sf", [P, T], F32, kind="ExternalInput")
    cm = nc.dram_tensor("cm", [P, 4 * TCH], BF16, kind="ExternalInput")
    oneb = nc.dram_tensor("oneb", [P, P], BF16, kind="ExternalInput")
    # out^T slice: [out_cols, b0 slice | b1 slice]
    outT = nc.dram_tensor("out", [DM, B * TSL], F32, kind="ExternalOutput")

    with tile.TileContext(nc) as tc:
        with tc.tile_pool(name="dram", bufs=1, space="DRAM") as dpool, \
             tc.tile_pool(name="persist", bufs=1) as keep:
            # A2A per batch: 8 rank-blocks x [256 local hd, 256 t-slice]
            ctxA_d = [dpool.tile([DM, TSL], BF16, name=f"ctxA{b}") for b in range(B)]
            gout_d = [dpool.tile([DM, TSL], BF16, name=f"gout{b}") for b in range(B)]

            # persistent SBUF: q/k (bf16, post-RoPE, [dims, keys]) and v
            # ([keys, j, d]) for every (batch, local-head) section
            qT_a = [[keep.tile([P, T], BF16, name=f"qTa{b}_{hl}")
                     for hl in range(HPC)] for b in range(B)]
            kT_a = [[keep.tile([P, T], BF16, name=f"kTa{b}_{hl}")
                     for hl in range(HPC)] for b in range(B)]
            v_a = [[keep.tile([P, NTB, D], BF16, name=f"va{b}_{hl}")
                    for hl in range(HPC)] for b in range(B)]
            cm_s = keep.tile([P, 4 * TCH], BF16)
            oneb_s = keep.tile([P, P], BF16)
            bar_in = dpool.tile([8, 4], F32)
            bar_out = dpool.tile([64, 4], F32, addr_space="Shared")
            # start-skew absorber: cores align here while phase 1 computes
            nc.sync.dma_start(bar_in[:], cf.ap()[0:8, 0:4])
            nc.gpsimd.collective_compute(
                "AllGather", mybir.AluOpType.bypass,
                replica_groups=[list(range(NCORES))],
                ins=[bar_in[:].opt()], outs=[bar_out[:].opt()])

            # ---------------- Phase 1: projections + RoPE ----------------
            with tc.tile_pool(name="p1w", bufs=1) as wpool, \
                 tc.tile_pool(name="p1", bufs=2) as pool, \
                 tc.tile_pool(name="psq", bufs=3, space="PSUM") as psqp, \
                 tc.tile_pool(name="psv", bufs=2, space="PSUM") as psvp:
                wq_s = wpool.tile([P, NKB, DLOC], BF16)
                wk_s = wpool.tile([P, NKB, DLOC], BF16)
                wv_s = wpool.tile([P, NKB, DLOC], BF16)
                cf_s = wpool.tile([P, T], F32)
                sf_s = wpool.tile([P, T], F32)
                nc.sync.dma_start(wq_s[:], wq.ap().rearrange("p (kb m) -> p kb m", kb=NKB))

                for i in range(BT // XCH):
                    bb, ic = i // (T // XCH), i % (T // XCH)
                    xt_t = pool.tile([P, NKB, XCH], BF16, tag="xt")
                    nc.scalar.dma_start(
                        xt_t[:], xt.ap()[i].rearrange("p (kb n) -> p kb n", kb=NKB))
                    if i == 0:
                        nc.sync.dma_start(cf_s[:], cf.ap())
                        nc.sync.dma_start(sf_s[:], sf.ap())
                        nc.scalar.dma_start(
                            wk_s[:], wk.ap().rearrange("p (kb m) -> p kb m", kb=NKB))
                        nc.scalar.dma_start(
                            wv_s[:], wv.ap().rearrange("p (kb m) -> p kb m", kb=NKB))
                        nc.sync.dma_start(cm_s[:], cm.ap())
                        nc.sync.dma_start(oneb_s[:], oneb.ap())
                    cs = cf_s[:, ic * XCH:(ic + 1) * XCH]
                    sn = sf_s[:, ic * XCH:(ic + 1) * XCH]
                    for w_s, dst in ((wq_s, qT_a), (wk_s, kT_a)):
                        for m in range(HPC):
                            ps = psqp.tile([P, XCH], F32, tag="qk")
                            for kb in range(NKB):
                                nc.tensor.matmul(
                                    ps[:], w_s[:, kb, m * P:(m + 1) * P],
                                    xt_t[:, kb],
                                    start=(kb == 0), stop=(kb == NKB - 1))
                            # RoPE: rq = q*cos_full + rot(q)*sin_signed,
                            # written straight into the persistent bf16 tile
                            tmp = pool.tile([P, XCH], F32, tag="tmp")
                            tmp2 = pool.tile([P, XCH], F32, tag="tmp2")
                            nc.vector.tensor_tensor(tmp[0:64], ps[64:128], sn[0:64], MUL)
                            nc.vector.tensor_tensor(tmp[64:128], ps[0:64], sn[64:128], MUL)
                            nc.vector.tensor_tensor(tmp2[:], ps[:], cs, MUL)
                            nc.vector.tensor_tensor(
                                dst[bb][m][:, ic * XCH:(ic + 1) * XCH],
                                tmp2[:], tmp[:], ADD)
                    for tb in range(XCH // P):
                        psv = psvp.tile([P, DLOC], F32, tag="v")
                        for kb in range(NKB):
                            nc.tensor.matmul(
                                psv[:], xt_t[:, kb, tb * P:(tb + 1) * P],
                                wv_s[:, kb],
                                start=(kb == 0), stop=(kb == NKB - 1))
                        jx = ic * (XCH // P) + tb
                        nc.vector.tensor_copy(v_a[bb][0][:, jx], psv[:, 0:D])
                        nc.vector.tensor_copy(v_a[bb][1][:, jx], psv[:, D:DLOC])

            # ------------- Phase 2: causal attention + per-batch A2A ---------
            with tc.tile_pool(name="p3w", bufs=1) as wpool3:
                g_ts = [wpool3.tile([P, NKB, TSL], BF16, name=f"g{b}_t")
                        for b in range(B)]
                with tc.tile_pool(name="p2", bufs=3) as pool2, \
                     tc.tile_pool(name="p2s", bufs=2) as spool, \
                     tc.tile_pool(name="p2t", bufs=6) as ppool, \
                     tc.tile_pool(name="ps_s", bufs=2, space="PSUM") as ps_sp, \
                     tc.tile_pool(name="ps_acc", bufs=2, space="PSUM") as ps_accp, \
                     tc.tile_pool(name="ps_bcp", bufs=2, space="PSUM") as ps_bcp:
                    wo_s = wpool3.tile([P, NKB, DM], BF16)
                    nc.sync.dma_start(
                        wo_s[:], wo.ap().rearrange("p (t m) -> p t m", t=NKB))

                    secs = [(b, hl) for b in range(B) for hl in range(HPC)]

                    def finalize(pend):
                        cq, S, ps_ctx, bq, hq = pend
                        ps_bc = ps_bcp.tile([P, TCH], F32, tag="bc")
                        nc.tensor.matmul(ps_bc[:], oneb_s[:], S[:, 0],
                                         start=True, stop=False)
                        nc.tensor.matmul(ps_bc[:], oneb_s[:], S[:, 1],
                                         start=False, stop=True)
                        bc_s = pool2.tile([P, TCH], F32, tag="bc_s")
                        nc.vector.reciprocal_approx_fast(bc_s[:], ps_bc[:])
                        ctx_s = pool2.tile([P, TCH], BF16, tag="ctx")
                        nc.vector.tensor_tensor(ctx_s[:], ps_ctx[:], bc_s[:], MUL)
                        nc.sync.dma_start(
                            ctxA_d[bq]
                            .rearrange("(r q p) n -> q p r n", q=HPC, p=P)
                            [hq, :, 2 * cq:2 * cq + 2],
                            ctx_s.rearrange("p (r n) -> p r n", r=2))

                    pend = None
                    for s, (b, hl) in enumerate(secs):
                        kT_s, v_s, qTF_s = kT_a[b][hl], v_a[b][hl], qT_a[b][hl]
                        first = (s == 0)
                        if s == 3:
                            # batch-0 gather: its A2A finished during section 3
                            nc.sync.dma_start(
                                g_ts[0][:],
                                gout_d[0][:].rearrange("(p t) n -> p t n", p=P))
                        for cq in range(NBCH):
                            nblk = 4 * cq + 4
                            qT_c = qTF_s[:, cq * TCH:(cq + 1) * TCH]
                            ps_ctx = ps_accp.tile([P, TCH], F32, tag="ctx")
                            S = spool.tile([P, 2, TCH], BF16, tag="S")
                            s_tiles = 0
                            pT0 = None
                            for jp in range(nblk // 2):
                                j0 = 2 * jp
                                ps_sc = ps_sp.tile([P, 2, TCH], F32, tag="s")
                                nc.tensor.matmul(
                                    ps_sc[:, 0], kT_s[:, j0 * P:(j0 + 1) * P],
                                    qT_c, start=True, stop=True)
                                nc.tensor.matmul(
                                    ps_sc[:, 1], kT_s[:, (j0 + 1) * P:(j0 + 2) * P],
                                    qT_c, start=True, stop=True)
                                pT = ppool.tile([P, 2, TCH], BF16, tag="pT")
                                vmask = j0 - 4 * cq
                                if vmask >= 0 and not first:
                                    # diagonal pair on warm tiles: skip the
                                    # all-masked column prefix of each block
                                    for h in range(2):
                                        off = (vmask + h) * P
                                        nc.scalar.activation(
                                            pT[:, h, off:], ps_sc[:, h, off:],
                                            EXP, scale=SCALE)
                                else:
                                    nc.scalar.activation(
                                        pT[:], ps_sc[:], EXP, scale=SCALE)
                                if vmask >= 0:
                                    # full-width: ctx matmuls read all columns,
                                    # so every masked entry must be zeroed
                                    nc.vector.tensor_tensor(
                                        pT[:], pT[:],
                                        cm_s[:, vmask * TCH:(vmask + 2) * TCH]
                                        .rearrange("p (v n) -> p v n", v=2), MUL)
                                # softmax-denominator partials, off the PE
                                s_tiles += 1
                                if s_tiles == 1:
                                    pT0 = pT
                                elif s_tiles == 2:
                                    nc.vector.tensor_tensor(S[:], pT0[:], pT[:], ADD)
                                elif vmask >= 0:
                                    c0 = vmask * P
                                    nc.vector.tensor_tensor(
                                        S[:, :, c0:], S[:, :, c0:],
                                        pT[:, :, c0:], ADD)
                                else:
                                    nc.vector.tensor_tensor(S[:], S[:], pT[:], ADD)
                                for h in range(2):
                                    j = j0 + h
                                    nc.tensor.matmul(
                                        ps_ctx[:], v_s[:, j], pT[:, h],
                                        start=(j == 0), stop=(j == nblk - 1))
                                if jp == 0 and pend is not None:
                                    finalize(pend)
                                    pend = None
                            pend = (cq, S, ps_ctx, b, hl)
                        if hl == HPC - 1:
                            finalize(pend)
                            pend = None
                            nc.gpsimd.collective_compute(
                                "AllToAll", mybir.AluOpType.bypass,
                                replica_groups=[list(range(NCORES))],
                                ins=[ctxA_d[b][:].opt()],
                                outs=[gout_d[b][:].opt()])

                # ---------------- Phase 3: output projection (full Wo) -----------
                with tc.tile_pool(name="p3", bufs=2) as pool3, \
                     tc.tile_pool(name="ps3", bufs=2, space="PSUM") as ps3:
                    for b in range(B):
                        g_t = g_ts[b]
                        if b == 1:
                            nc.sync.dma_start(
                                g_t[:],
                                gout_d[1][:].rearrange("(p t) n -> p t n", p=P))
                        for m in range(DM // P):
                            pso = ps3.tile([P, TSL], F32, tag="o")
                            for t in range(NKB):
                                nc.tensor.matmul(
                                    pso[:], wo_s[:, t, m * P:(m + 1) * P], g_t[:, t],
                                    start=(t == 0), stop=(t == NKB - 1))
                            o_s = pool3.tile([P, TSL], F32, tag="o_s")
                            nc.vector.tensor_copy(o_s[:], pso[:])
                            nc.sync.dma_start(
                                outT.ap()[m * P:(m + 1) * P, b * TSL:(b + 1) * TSL],
                                o_s[:])

    nc.compile()
    return nc


def _prep_inputs(x, cos, sin, Wq, Wk, Wv, Wo):
    x = np.asarray(x, dtype=np.float32)
    cos = np.asarray(cos, dtype=np.float32)
    sin = np.asarray(sin, dtype=np.float32)
    # xt packed: [chunk, partition, kb*XCH] so each chunk load is 128
    # contiguous 16KB descriptors
    xt2 = np.ascontiguousarray(x.reshape(BT, DM).T)          # [DM, BT]
    xtp = np.ascontiguousarray(
        xt2.reshape(NKB, P, BT // XCH, XCH).transpose(2, 1, 0, 3)
        .reshape(BT // XCH, P, NKB * XCH)).astype(ml_dtypes.bfloat16)
    cf = np.empty((P, T), np.float32)
    cf[:64] = cos.T
    cf[64:] = cos.T
    sf = np.empty((P, T), np.float32)
    sf[:64] = -sin.T
    sf[64:] = sin.T
    qq = np.arange(TCH, dtype=np.int64)[None, :]
    rr = np.arange(P, dtype=np.int64)[:, None]
    cm = np.concatenate(
        [(qq >= v * P + rr).astype(np.float32) for v in range(TCH // P)],
        axis=1).astype(ml_dtypes.bfloat16)
    oneb = np.ones((P, P), np.float32).astype(ml_dtypes.bfloat16)

    def pack_w(w):  # [DM, M] -> [P, NKB*M], contraction-block-major
        w = np.asarray(w, np.float32)
        m = w.shape[1]
        return np.ascontiguousarray(
            w.reshape(NKB, P, m).transpose(1, 0, 2).reshape(P, NKB * m)
        ).astype(ml_dtypes.bfloat16)

    # wo packed in gather-slot order: contraction group t = rows {16p+t},
    # matching the linear [128 x 8KB] gather of the A2A output
    wo_p = np.ascontiguousarray(
        np.asarray(Wo, np.float32).reshape(P, NKB, DM).reshape(P, NKB * DM)
    ).astype(ml_dtypes.bfloat16)
    in_maps = []
    for c in range(NCORES):
        sl = slice(c * DLOC, (c + 1) * DLOC)
        in_maps.append({
            "xt": xtp, "cf": cf, "sf": sf, "cm": cm, "oneb": oneb,
            "wq": pack_w(np.asarray(Wq, np.float32)[:, sl]),
            "wk": pack_w(np.asarray(Wk, np.float32)[:, sl]),
            "wv": pack_w(np.asarray(Wv, np.float32)[:, sl]),
            "wo": wo_p,
        })
    return in_maps


def run(x, mask, cos, sin, Wq, Wk, Wv, Wo, trace=False):
    global _nc_cache
    if _nc_cache is None:
        _nc_cache = _build()
    in_maps = _prep_inputs(x, cos, sin, Wq, Wk, Wv, Wo)
    res = bass_utils.run_bass_kernel_spmd(
        _nc_cache, in_maps, core_ids=list(range(NCORES)), trace=trace)
    out = np.empty((B, T, DM), np.float32)
    for c in range(NCORES):
        o = res.results[c]["out"]  # [DM, B*TSL]
        for b in range(B):
            out[b, c * TSL:(c + 1) * TSL, :] = o[:, b * TSL:(b + 1) * TSL].T
    return out, res


def kernel(x, mask, cos, sin, Wq, Wk, Wv, Wo):
    out, _ = run(x, mask, cos, sin, Wq, Wk, Wv, Wo, trace=False)
    return out


# revision 15
# speedup vs baseline: 1.4227x; 1.0630x over previous
"""Multi-head causal attention with RoPE on 8 TRN2 NeuronCores.

Tensor-parallel over heads: core c computes heads (2c, 2c+1).
  Phase 1: Q^T,K^T (with RoPE) and V projections from host-packed bf16
           x/weights.  Q^T/K^T (bf16, post-RoPE) and V (bf16) are written
           DIRECTLY into persistent SBUF tiles — no DRAM roundtrip, so
           phase 2 needs no loads at all.
  Phase 2: causal attention per (batch, head), scores^T = K^T_blk^T @ Q^T,
           softmax without max-subtraction.  Softmax denominators: exp
           tiles are accumulated elementwise (bf16) on the Vector engine,
           then two ones-matmuls per 512-query chunk reduce the halves
           over the partition axis directly into a broadcast [128,512]
           PSUM tile; 1/x uses the fast custom-DVE approximation (plain
           InstReciprocal costs ~4us fixed).  The denominator / normalize
           / scatter stage of chunk n is emitted inside chunk n+1's first
           key-pair (one-chunk software pipeline) so the PE and the ctx
           PSUM pool never wait on it.
  Phase 3: per-batch AllToAll (2 collectives; the batch-0 one hides under
           batch-1 attention, the batch-1 one under batch-0's Wo)
           redistributes context from head-sharded to sequence-sharded;
           each core applies the FULL Wo to its 256-token slice per
           batch.  Wo is host-packed in gather-slot order (row u=16p+t ->
           [p,t]) so the post-collective gather is a single linear DMA
           (128 contiguous 8KB descriptors) instead of 2048 512B ones.
All bulk inputs are host-packed so each DMA is 128 large descriptors
(DMA trigger time is proportional to descriptor count and blocks the
issuing engine's sequencer).  Sync HWDGE queue: weights/consts, ctx
scatters, gathers, outputs.  Activation HWDGE queue: x chunks.
"""
import ml_dtypes
import numpy as np

import concourse.bass as bass  # noqa: F401  (engine namespaces live on nc)
import concourse.mybir as mybir
import concourse.tile as tile
from concourse import bacc
from concourse import bass_utils

B, T, DM, H, D = 2, 2048, 2048, 16, 128
NCORES = 8
HPC = H // NCORES        # heads per core
DLOC = HPC * D           # local head width (256)
BT = B * T               # 4096 token rows
P = 128
TCH = 512                # free-dim chunk
XCH = 512                # phase-1 token chunk
NKB = DM // P            # 16 contraction blocks
NTB = T // P             # 16 token blocks per batch
NBCH = T // TCH          # 4 token chunks per batch
TSL = T // NCORES        # 256-token output slice per core per batch
SCALE = 1.0 / float(np.sqrt(D))
F32 = mybir.dt.float32
BF16 = mybir.dt.bfloat16
MUL = mybir.AluOpType.mult
ADD = mybir.AluOpType.add
EXP = mybir.ActivationFunctionType.Exp

_nc_cache = None


def _build():
    nc = bacc.Bacc("TRN2", target_bir_lowering=False, debug=False,
                   num_devices=NCORES)
    xt = nc.dram_tensor("xt", [BT // XCH, P, NKB * XCH], BF16, kind="ExternalInput")
    wq = nc.dram_tensor("wq", [P, NKB * DLOC], BF16, kind="ExternalInput")
    wk = nc.dram_tensor("wk", [P, NKB * DLOC], BF16, kind="ExternalInput")
    wv = nc.dram_tensor("wv", [P, NKB * DLOC], BF16, kind="ExternalInput")
    wo = nc.dram_tensor("wo", [P, NKB * DM], BF16, kind="ExternalInput")
    cf = nc.dram_tensor("cf", [P, T], F32, kind="ExternalInput")
    sf = nc.dram_tensor("sf", [P, T], F32, kind="ExternalInput")
    cm = nc.dram_tensor("cm", [P, 4 * TCH], BF16, kind="ExternalInput")
    oneb = nc.dram_tensor("oneb", [P, P], BF16, kind="ExternalInput")
    # out^T slice: [out_cols, b0 slice | b1 slice]
    outT = nc.dram_tensor("out", [DM, B * TSL], F32, kind="ExternalOutput")

    with tile.TileContext(nc) as tc:
        with tc.tile_pool(name="dram", bufs=1, space="DRAM") as dpool, \
             tc.tile_pool(name="persist", bufs=1) as keep:
            # A2A per batch: 8 rank-blocks x [256 local hd, 256 t-slice]
            ctxA_d = [dpool.tile([DM, TSL], BF16, name=f"ctxA{b}") for b in range(B)]
            gout_d = [dpool.tile([DM, TSL], BF16, name=f"gout{b}") for b in range(B)]

            # persistent SBUF: q/k (bf16, post-RoPE, [dims, keys]) and v
            # ([keys, j, d]) for every (batch, local-head) section
            qT_a = [[keep.tile([P, T], BF16, name=f"qTa{b}_{hl}")
                     for hl in range(HPC)] for b in range(B)]
            kT_a = [[keep.tile([P, T], BF16, name=f"kTa{b}_{hl}")
                     for hl in range(HPC)] for b in range(B)]
            v_a = [[keep.tile([P, NTB, D], BF16, name=f"va{b}_{hl}")
                    for hl in range(HPC)] for b in range(B)]
            cm_s = keep.tile([P, 4 * TCH], BF16)
            oneb_s = keep.tile([P, P], BF16)
            bar_in = dpool.tile([8, 4], F32)
            bar_out = dpool.tile([64, 4], F32, addr_space="Shared")
            # start-skew absorber: cores align here while phase 1 computes
            nc.sync.dma_start(bar_in[:], cf.ap()[0:8, 0:4])
            nc.gpsimd.collective_compute(
                "AllGather", mybir.AluOpType.bypass,
                replica_groups=[list(range(NCORES))],
                ins=[bar_in[:].opt()], outs=[bar_out[:].opt()])

            # ---------------- Phase 1: projections + RoPE ----------------
            with tc.tile_pool(name="p1w", bufs=1) as wpool, \
                 tc.tile_pool(name="p1", bufs=2) as pool, \
                 tc.tile_pool(name="psq", bufs=3, space="PSUM") as psqp, \
                 tc.tile_pool(name="psv", bufs=2, space="PSUM") as psvp:
                wq_s = wpool.tile([P, NKB, DLOC], BF16)
                wk_s = wpool.tile([P, NKB, DLOC], BF16)
                wv_s = wpool.tile([P, NKB, DLOC], BF16)
                cf_s = wpool.tile([P, T], F32)
                sf_s = wpool.tile([P, T], F32)
                nc.sync.dma_start(wq_s[:], wq.ap().rearrange("p (kb m) -> p kb m", kb=NKB))

                for i in range(BT // XCH):
                    bb, ic = i // (T // XCH), i % (T // XCH)
                    xt_t = pool.tile([P, NKB, XCH], BF16, tag="xt")
                    nc.scalar.dma_start(
                        xt_t[:], xt.ap()[i].rearrange("p (kb n) -> p kb n", kb=NKB))
                    if i == 0:
                        nc.sync.dma_start(cf_s[:], cf.ap())
                        nc.sync.dma_start(sf_s[:], sf.ap())
                        nc.scalar.dma_start(
                            wk_s[:], wk.ap().rearrange("p (kb m) -> p kb m", kb=NKB))
                        nc.scalar.dma_start(
                            wv_s[:], wv.ap().rearrange("p (kb m) -> p kb m", kb=NKB))
                        nc.sync.dma_start(cm_s[:], cm.ap())
                        nc.sync.dma_start(oneb_s[:], oneb.ap())
                    cs = cf_s[:, ic * XCH:(ic + 1) * XCH]
                    sn = sf_s[:, ic * XCH:(ic + 1) * XCH]
                    for w_s, dst in ((wq_s, qT_a), (wk_s, kT_a)):
                        for m in range(HPC):
                            ps = psqp.tile([P, XCH], F32, tag="qk")
                            for kb in range(NKB):
                                nc.tensor.matmul(
                                    ps[:], w_s[:, kb, m * P:(m + 1) * P],
                                    xt_t[:, kb],
                                    start=(kb == 0), stop=(kb == NKB - 1))
                            # RoPE: rq = q*cos_full + rot(q)*sin_signed,
                            # written straight into the persistent bf16 tile
                            tmp = pool.tile([P, XCH], F32, tag="tmp")
                            tmp2 = pool.tile([P, XCH], F32, tag="tmp2")
                            nc.vector.tensor_tensor(tmp[0:64], ps[64:128], sn[0:64], MUL)
                            nc.vector.tensor_tensor(tmp[64:128], ps[0:64], sn[64:128], MUL)
                            nc.vector.tensor_tensor(tmp2[:], ps[:], cs, MUL)
                            nc.vector.tensor_tensor(
                                dst[bb][m][:, ic * XCH:(ic + 1) * XCH],
                                tmp2[:], tmp[:], ADD)
                    for tb in range(XCH // P):
                        psv = psvp.tile([P, DLOC], F32, tag="v")
                        for kb in range(NKB):
                            nc.tensor.matmul(
                                psv[:], xt_t[:, kb, tb * P:(tb + 1) * P],
                                wv_s[:, kb],
                                start=(kb == 0), stop=(kb == NKB - 1))
                        jx = ic * (XCH // P) + tb
                        nc.vector.tensor_copy(v_a[bb][0][:, jx], psv[:, 0:D])
                        nc.vector.tensor_copy(v_a[bb][1][:, jx], psv[:, D:DLOC])

            # ------------- Phase 2: causal attention + per-batch A2A ---------
            with tc.tile_pool(name="p3w", bufs=1) as wpool3:
                g_ts = [wpool3.tile([P, NKB, TSL], BF16, name=f"g{b}_t")
                        for b in range(B)]
                with tc.tile_pool(name="p2", bufs=3) as pool2, \
                     tc.tile_pool(name="p2s", bufs=2) as spool, \
                     tc.tile_pool(name="p2t", bufs=6) as ppool, \
                     tc.tile_pool(name="ps_s", bufs=2, space="PSUM") as ps_sp, \
                     tc.tile_pool(name="ps_acc", bufs=2, space="PSUM") as ps_accp, \
                     tc.tile_pool(name="ps_bcp", bufs=2, space="PSUM") as ps_bcp:
                    wo_s = wpool3.tile([P, NKB, DM], BF16)
                    nc.sync.dma_start(
                        wo_s[:], wo.ap().rearrange("p (t m) -> p t m", t=NKB))

                    secs = [(b, hl) for b in range(B) for hl in range(HPC)]

                    def finalize(pend):
                        cq, S, ps_ctx, bq, hq = pend
                        ps_bc = ps_bcp.tile([P, TCH], F32, tag="bc")
                        nc.tensor.matmul(ps_bc[:], oneb_s[:], S[:, 0],
                                         start=True, stop=False)
                        nc.tensor.matmul(ps_bc[:], oneb_s[:], S[:, 1],
                                         start=False, stop=True)
                        bc_s = pool2.tile([P, TCH], F32, tag="bc_s")
                        nc.vector.reciprocal_approx_fast(bc_s[:], ps_bc[:])
                        ctx_s = pool2.tile([P, TCH], BF16, tag="ctx")
                        nc.vector.tensor_tensor(ctx_s[:], ps_ctx[:], bc_s[:], MUL)
                        nc.sync.dma_start(
                            ctxA_d[bq]
                            .rearrange("(r q p) n -> q p r n", q=HPC, p=P)
                            [hq, :, 2 * cq:2 * cq + 2],
                            ctx_s.rearrange("p (r n) -> p r n", r=2))

                    pend = None
                    for s, (b, hl) in enumerate(secs):
                        kT_s, v_s, qTF_s = kT_a[b][hl], v_a[b][hl], qT_a[b][hl]
                        first = (s == 0)
                        if s == 3:
                            # batch-0 gather: its A2A finished during section 3
                            nc.sync.dma_start(
                                g_ts[0][:],
                                gout_d[0][:].rearrange("(p t) n -> p t n", p=P))
                        for cq in range(NBCH):
                            nblk = 4 * cq + 4
                            qT_c = qTF_s[:, cq * TCH:(cq + 1) * TCH]
                            ps_ctx = ps_accp.tile([P, TCH], F32, tag="ctx")
                            S = spool.tile([P, 2, TCH], BF16, tag="S")
                            s_tiles = 0
                            pT0 = None
                            for jp in range(nblk // 2):
                                j0 = 2 * jp
                                ps_sc = ps_sp.tile([P, 2, TCH], F32, tag="s")
                                vm0 = j0 - 4 * cq
                                for h in range(2):
                                    # diagonal blocks: skip the all-masked
                                    # column prefix (warm PSUM only)
                                    off = ((vm0 + h) * P
                                           if vm0 >= 0 and not first else 0)
                                    nc.tensor.matmul(
                                        ps_sc[:, h, off:],
                                        kT_s[:, (j0 + h) * P:(j0 + h + 1) * P],
                                        qT_c[:, off:], start=True, stop=True)
                                pT = ppool.tile([P, 2, TCH], BF16, tag="pT")
                                vmask = j0 - 4 * cq
                                if vmask >= 0 and not first:
                                    # diagonal pair on warm tiles: skip the
                                    # all-masked column prefix of each block
                                    for h in range(2):
                                        off = (vmask + h) * P
                                        nc.scalar.activation(
                                            pT[:, h, off:], ps_sc[:, h, off:],
                                            EXP, scale=SCALE)
                                else:
                                    nc.scalar.activation(
                                        pT[:], ps_sc[:], EXP, scale=SCALE)
                                if vmask >= 0:
                                    # full-width: ctx matmuls read all columns,
                                    # so every masked entry must be zeroed
                                    nc.vector.tensor_tensor(
                                        pT[:], pT[:],
                                        cm_s[:, vmask * TCH:(vmask + 2) * TCH]
                                        .rearrange("p (v n) -> p v n", v=2), MUL)
                                # softmax-denominator partials, off the PE
                                s_tiles += 1
                                if s_tiles == 1:
                                    pT0 = pT
                                elif s_tiles == 2:
                                    nc.vector.tensor_tensor(S[:], pT0[:], pT[:], ADD)
                                elif vmask >= 0:
                                    c0 = vmask * P
                                    nc.vector.tensor_tensor(
                                        S[:, :, c0:], S[:, :, c0:],
                                        pT[:, :, c0:], ADD)
                                else:
                                    nc.vector.tensor_tensor(S[:], S[:], pT[:], ADD)
                                for h in range(2):
                                    j = j0 + h
                                    nc.tensor.matmul(
                                        ps_ctx[:], v_s[:, j], pT[:, h],
                                        start=(j == 0), stop=(j == nblk - 1))
                                if jp == 0 and pend is not None:
                                    finalize(pend)
                                    pend = None
                            pend = (cq, S, ps_ctx, b, hl)
                        if hl == HPC - 1:
                            finalize(pend)
                            pend = None
                            nc.gpsimd.collective_compute(
                                "AllToAll", mybir.AluOpType.bypass,
                                replica_groups=[list(range(NCORES))],
                                ins=[ctxA_d[b][:].opt()],
                                outs=[gout_d[b][:].opt()])

                # ---------------- Phase 3: output projection (full Wo) -----------
                with tc.tile_pool(name="p3", bufs=4) as pool3, \
                     tc.tile_pool(name="ps3", bufs=4, space="PSUM") as ps3:
                    for b in range(B):
                        g_t = g_ts[b]
                        if b == 1:
                            nc.sync.dma_start(
                                g_t[:],
                                gout_d[1][:].rearrange("(p t) n -> p t n", p=P))
                        for m in range(DM // P):
                            pso = ps3.tile([P, TSL], F32, tag="o")
                            for t in range(NKB):
                                nc.tensor.matmul(
                                    pso[:], wo_s[:, t, m * P:(m + 1) * P], g_t[:, t],
                                    start=(t == 0), stop=(t == NKB - 1))
                            o_s = pool3.tile([P, TSL], F32, tag="o_s")
                            nc.scalar.copy(o_s[:], pso[:])
                            nc.sync.dma_start(
                                outT.ap()[m * P:(m + 1) * P, b * TSL:(b + 1) * TSL],
                                o_s[:])

    nc.compile()
    return nc


def _prep_inputs(x, cos, sin, Wq, Wk, Wv, Wo):
    x = np.asarray(x, dtype=np.float32)
    cos = np.asarray(cos, dtype=np.float32)
    sin = np.asarray(sin, dtype=np.float32)
    # xt packed: [chunk, partition, kb*XCH] so each chunk load is 128
    # contiguous 16KB descriptors
    xt2 = np.ascontiguousarray(x.reshape(BT, DM).T)          # [DM, BT]
    xtp = np.ascontiguousarray(
        xt2.reshape(NKB, P, BT // XCH, XCH).transpose(2, 1, 0, 3)
        .reshape(BT // XCH, P, NKB * XCH)).astype(ml_dtypes.bfloat16)
    cf = np.empty((P, T), np.float32)
    cf[:64] = cos.T
    cf[64:] = cos.T
    sf = np.empty((P, T), np.float32)
    sf[:64] = -sin.T
    sf[64:] = sin.T
    qq = np.arange(TCH, dtype=np.int64)[None, :]
    rr = np.arange(P, dtype=np.int64)[:, None]
    cm = np.concatenate(
        [(qq >= v * P + rr).astype(np.float32) for v in range(TCH // P)],
        axis=1).astype(ml_dtypes.bfloat16)
    oneb = np.ones((P, P), np.float32).astype(ml_dtypes.bfloat16)

    def pack_w(w):  # [DM, M] -> [P, NKB*M], contraction-block-major
        w = np.asarray(w, np.float32)
        m = w.shape[1]
        return np.ascontiguousarray(
            w.reshape(NKB, P, m).transpose(1, 0, 2).reshape(P, NKB * m)
        ).astype(ml_dtypes.bfloat16)

    # wo packed in gather-slot order: contraction group t = rows {16p+t},
    # matching the linear [128 x 8KB] gather of the A2A output
    wo_p = np.ascontiguousarray(
        np.asarray(Wo, np.float32).reshape(P, NKB, DM).reshape(P, NKB * DM)
    ).astype(ml_dtypes.bfloat16)
    in_maps = []
    for c in range(NCORES):
        sl = slice(c * DLOC, (c + 1) * DLOC)
        in_maps.append({
            "xt": xtp, "cf": cf, "sf": sf, "cm": cm, "oneb": oneb,
            "wq": pack_w(np.asarray(Wq, np.float32)[:, sl]),
            "wk": pack_w(np.asarray(Wk, np.float32)[:, sl]),
            "wv": pack_w(np.asarray(Wv, np.float32)[:, sl]),
            "wo": wo_p,
        })
    return in_maps


def run(x, mask, cos, sin, Wq, Wk, Wv, Wo, trace=False):
    global _nc_cache
    if _nc_cache is None:
        _nc_cache = _build()
    in_maps = _prep_inputs(x, cos, sin, Wq, Wk, Wv, Wo)
    res = bass_utils.run_bass_kernel_spmd(
        _nc_cache, in_maps, core_ids=list(range(NCORES)), trace=trace)
    out = np.empty((B, T, DM), np.float32)
    for c in range(NCORES):
        o = res.results[c]["out"]  # [DM, B*TSL]
        for b in range(B):
            out[b, c * TSL:(c + 1) * TSL, :] = o[:, b * TSL:(b + 1) * TSL].T
    return out, res


def kernel(x, mask, cos, sin, Wq, Wk, Wv, Wo):
    out, _ = run(x, mask, cos, sin, Wq, Wk, Wv, Wo, trace=False)
    return out
